# revision 1
# baseline (speedup 1.0000x reference)
"""Trainium2 Bass kernel for nn_LEAP_74371653697613 (GRU decoder w/ additive attention).

Structure exploited:
  - softmax(ctx_score + h.w_h + b) == softmax(ctx_score): attention weights are
    constant across decode steps -> context vector c computed once on device.
  - gi_t = W_ih @ [c; x_t] + b_ih is teacher-forced -> batched matmuls, precomputed.
  - logits don't feed back -> one big relu(H) @ out_w^T matmul at the end,
    vocab-sharded across the 8 cores (each core gets a 4096-row slice of out_w).
  - only W_hh @ h_t + gates is sequential (65 steps); it runs identically on all
    8 cores (replicated -> zero cross-core communication).

Per-step device schedule:
  4 col-tiled groups of fp32 matvec MMs (h chunks stationary, W_hh^T moving) +
  a one-hot K=65 matmul folding gi_rz[t] into the same PSUM accumulation;
  bridge PSUM rows {0,32,64,96} -> SBUF via 2 ACT + 2 DVE copies; compact to
  contiguous partitions with a selector matmul; gates on [4, 256] tiles;
  h_stat rebuilt with 2 PE transposes.
"""
import os
import sys
import numpy as np

for _p in ("/opt/trn_rl_repo", "/root/.axon_site/_ro/trn_rl_repo"):
    if os.path.isdir(_p) and _p not in sys.path:
        sys.path.insert(0, _p)

import concourse.bass as bass
import concourse.bacc as bacc
import concourse.tile as tile
import concourse.mybir as mybir
from concourse.bass_utils import run_bass_kernel_spmd
from concourse.masks import make_identity

F32 = mybir.dt.float32
BF16 = mybir.dt.bfloat16
AF = mybir.ActivationFunctionType
ALU = mybir.AluOpType
NP_BF16 = mybir.dt.np(BF16)

E = 1024          # emb dim
EC = 8            # E / 128 chunks
T = 65            # decode steps (1 SOS + 64)
L = 320           # context rows (128 + 64 + 128)
V0 = 32000
V = V0 + 2        # 32002
NCORES = 8
VP = 4096         # per-core padded vocab slice (8 * 4096 = 32768 >= 32002)
G = 4             # col-tile groups
RG = 768          # region width per group (3 gates x 256)

_CACHE = {}


def _region_rows(w):
    """Reorder gate rows [3072, X] into region order: region (j, g*256+mm)
    <- row g*1024 + j*256 + mm."""
    x = w.reshape(3, 4, 256, -1)                 # g, j, mm, rest
    return np.ascontiguousarray(np.transpose(x, (1, 0, 2, 3))).reshape(3072, -1)


def _arrange_w(w):
    """[3072, 1024] -> [128, 8*4*768]: out[p, ((c*4)+j)*768 + g*256+mm]
    = w[g*1024 + j*256 + mm, c*128 + p]."""
    x = w.reshape(3, 4, 256, EC, 128)            # g, j, mm, c, p
    x = np.transpose(x, (4, 3, 1, 0, 2))         # p, c, j, g, mm
    return np.ascontiguousarray(x).reshape(128, EC * G * RG)


def _bias_tall(b_rzn):
    """[3072] bias in gate order -> [128, 768] with row 32j = region j."""
    x = b_rzn.reshape(3, 4, 256)                 # g, j, mm
    x = np.transpose(x, (1, 0, 2)).reshape(4, RG)  # j, (g mm)
    out = np.zeros((128, RG), np.float32)
    out[::32, :] = x
    return out


def build_program(rec_steps=T, do_final=True, do_gi=True, do_ph1=True, do_gic=True, do_wxdma=True):
    nc = bacc.Bacc("TRN2", target_bir_lowering=False, debug=False, num_devices=NCORES)

    ctx_d = nc.dram_tensor("ctx", [L, E], F32, kind="ExternalInput").ap()
    decx_d = nc.dram_tensor("decx", [T, E], F32, kind="ExternalInput").ap()
    we_d = nc.dram_tensor("we", [1, E], F32, kind="ExternalInput").ap()
    whh_d = nc.dram_tensor("whh", [128, EC * G * RG], F32, kind="ExternalInput").ap()
    wc_d = nc.dram_tensor("wc", [128, EC * G * RG], BF16, kind="ExternalInput").ap()
    wx_d = nc.dram_tensor("wx", [128, EC * G * RG], BF16, kind="ExternalInput").ap()
    bias_d = nc.dram_tensor("bias", [128, RG], F32, kind="ExternalInput").ap()
    owt_d = nc.dram_tensor("owt", [128, EC * VP], BF16, kind="ExternalInput").ap()
    outb_d = nc.dram_tensor("outb", [1, VP], F32, kind="ExternalInput").ap()
    out_d = nc.dram_tensor("out", [T, VP], F32, kind="ExternalOutput").ap()

    with tile.TileContext(nc) as tc:
        with tc.tile_pool(name="persist", bufs=1) as pp:
            # ---------- persistent constants ----------
            whh = pp.tile([128, EC * G * RG], F32)
            nc.sync.dma_start(whh[:], whh_d[:])

            ident = pp.tile([128, 128], F32)
            make_identity(nc, ident[:])
            ident_bf = pp.tile([128, 128], BF16)
            nc.vector.tensor_copy(ident_bf[:], ident[:])

            sel4 = pp.tile([128, 4], F32)          # sel4[k, m] = 1[k == 32m]
            nc.gpsimd.memset(sel4[:], 0.0)
            nc.gpsimd.affine_select(out=sel4[:], in_=sel4[:], compare_op=ALU.not_equal,
                                    fill=1.0, base=0, pattern=[[-32, 4]],
                                    channel_multiplier=1)

            ones_tall = pp.tile([128, T], F32)
            nc.gpsimd.memset(ones_tall[:], 1.0)
            ones_col = pp.tile([128, 1], F32)
            nc.gpsimd.memset(ones_col[:], 1.0)
            ones_row = pp.tile([1, 128], F32)
            nc.gpsimd.memset(ones_row[:], 1.0)

            bias_tall = pp.tile([128, RG], F32)    # (b_ih + b_hh) rz | b_ih n, rows at 32j
            nc.sync.dma_start(bias_tall[:], bias_d[:])

            girz = pp.tile([T, G, 512], BF16)      # gi rz-part, partition = t
            gin = pp.tile([4, T * 256], BF16)      # gi n-part, [j, t*256+m]
            gic_tall = pp.tile([128, RG], F32)     # const part of gi, rows at 32j
            ht_full = pp.tile([128, EC * T], BF16)  # [p, c, t] = relu(h_t[c*128+p])
            h_stat = pp.tile([128, EC], F32)
            h_rows = pp.tile([4, 256], F32)
            cT_bf = pp.tile([128, EC], BF16)
            dxT_bf = pp.tile([128, EC, T], BF16)
            # bridge buffers: only rows {32j} are written per step, but the
            # selector matmul streams all 128 partitions -> zero-init once so
            # unwritten rows are 0.0 (not stale/NaN bytes).
            gtallA = pp.tile([128, RG], F32)
            gtallB = pp.tile([128, RG], F32)
            nc.gpsimd.memset(gtallA[:], 0.0)
            nc.gpsimd.memset(gtallB[:], 0.0)

            # h0 = dec_emb[SOS] = decx row 0, in both layouts
            nc.sync.dma_start(h_stat[:], decx_d[0:1, :].rearrange("o (c p) -> (o p) c", p=128))
            nc.sync.dma_start(h_rows[:], decx_d[0:1, :].rearrange("o (j m) -> (o j) m", j=4))

            # ---------- phase 1: attention (constant across steps) ----------
            if do_ph1:
                with tc.tile_pool(name="ph1", bufs=1) as p1, \
                     tc.tile_pool(name="ph1ps", bufs=1, space="PSUM") as p1ps:
                    we_sb = p1.tile([1, E], F32)
                    nc.sync.dma_start(we_sb[:], we_d[:])
                    rows3 = (128, 128, 64)
                    ctx_sb = []
                    for i, rows in enumerate(rows3):
                        t_ = p1.tile([128, E], F32, tag=f"ctx{i}")
                        nc.sync.dma_start(t_[:rows, :], ctx_d[128 * i:128 * i + rows, :])
                        ctx_sb.append(t_)
                    decx_sb = p1.tile([T, E], F32)
                    nc.sync.dma_start(decx_sb[:], decx_d[:])

                    # replicate w_e across partitions via K=1 matmul
                    werep_ps = p1ps.tile([128, E], F32, space="PSUM")
                    for half in range(2):
                        nc.tensor.matmul(werep_ps[:, 512 * half:512 * (half + 1)],
                                         lhsT=ones_row[:1, :],
                                         rhs=we_sb[:1, 512 * half:512 * (half + 1)],
                                         start=True, stop=True)
                    werep = p1.tile([128, E], F32)
                    nc.vector.tensor_copy(werep[:], werep_ps[:])

                    scratch = p1.tile([128, E], F32)
                    escore = [p1.tile([128, 1], F32, tag=f"esc{i}", name=f"esc{i}")
                              for i in range(3)]
                    for i, rows in enumerate(rows3):
                        sc = p1.tile([128, 1], F32, tag=f"sc{i}")
                        nc.vector.tensor_tensor(out=scratch[:rows, :],
                                                in0=ctx_sb[i][:rows, :],
                                                in1=werep[:rows, :], op=ALU.mult)
                        nc.vector.tensor_reduce(out=sc[:rows, :], in_=scratch[:rows, :],
                                                axis=mybir.AxisListType.X, op=ALU.add)
                        nc.scalar.activation(escore[i][:rows, :], sc[:rows, :], AF.Exp)
                    ssum_ps = p1ps.tile([1, 1], F32, space="PSUM")
                    for i, rows in enumerate(rows3):
                        nc.tensor.matmul(ssum_ps[:1, :1], lhsT=escore[i][:rows, :1],
                                         rhs=ones_col[:rows, :1],
                                         start=(i == 0), stop=(i == 2))
                    rsum = p1.tile([1, 1], F32)
                    nc.vector.reciprocal(rsum[:], ssum_ps[:1, :1])

                    cun_ps = p1ps.tile([1, E], F32, space="PSUM")
                    for half in range(2):
                        for i, rows in enumerate(rows3):
                            nc.tensor.matmul(cun_ps[:1, 512 * half:512 * (half + 1)],
                                             lhsT=escore[i][:rows, :1],
                                             rhs=ctx_sb[i][:rows, 512 * half:512 * (half + 1)],
                                             start=(i == 0), stop=(i == 2))
                    c_sb = p1.tile([1, E], F32)
                    nc.vector.tensor_scalar_mul(c_sb[:], cun_ps[:1, :], rsum[:1, :1])

                    # c^T [128, 8] bf16 via PE transposes
                    cT_ps = p1ps.tile([128, EC], F32, space="PSUM")
                    for k in range(EC):
                        nc.tensor.transpose(out=cT_ps[:, k:k + 1],
                                            in_=c_sb[:1, 128 * k:128 * (k + 1)],
                                            identity=ident[:1, :1])
                    nc.vector.tensor_copy(cT_bf[:], cT_ps[:])

                    # dec_x^T [128, 8, 65] bf16 via PE transposes
                    dxT_ps = p1ps.tile([128, T], F32, space="PSUM")
                    for k in range(EC):
                        nc.tensor.transpose(out=dxT_ps[:, :],
                                            in_=decx_sb[:T, 128 * k:128 * (k + 1)],
                                            identity=ident[:T, :T])
                        nc.vector.tensor_copy(dxT_bf[:, k, :], dxT_ps[:, :])

            # ---------- phase 2: gic = W_ih[:, :E] @ c + biases ----------
            with tc.tile_pool(name="pwc", bufs=1) as pwc, \
                 tc.tile_pool(name="pwcps", bufs=1, space="PSUM") as pwcps:
                wc_sb = pwc.tile([128, EC * G * RG], BF16)
                nc.sync.dma_start(wc_sb[:], wc_d[:])
                wcv = wc_sb[:].rearrange("p (c j m) -> p c j m", c=EC, j=G)
                gic_ps = pwcps.tile([128, 1024], F32, space="PSUM")
                for j in range(G if do_gic else 0):
                    for c in range(EC):
                        nc.tensor.matmul(gic_ps[32 * j:32 * j + 1, 0:512],
                                         lhsT=cT_bf[:, c:c + 1], rhs=wcv[:, c, j, 0:512],
                                         start=(c == 0), stop=False,
                                         tile_position=(0, 32 * j))
                        nc.tensor.matmul(gic_ps[32 * j:32 * j + 1, 512:768],
                                         lhsT=cT_bf[:, c:c + 1], rhs=wcv[:, c, j, 512:768],
                                         start=(c == 0), stop=False,
                                         tile_position=(0, 32 * j))
                    # + (b_ih + b_hh)_rz | b_ih_n  via K=1 matmul
                    nc.tensor.matmul(gic_ps[32 * j:32 * j + 1, 0:512],
                                     lhsT=ones_tall[32 * j:32 * j + 1, 0:1],
                                     rhs=bias_tall[32 * j:32 * j + 1, 0:512],
                                     start=False, stop=True,
                                     tile_position=(32 * j, 32 * j))
                    nc.tensor.matmul(gic_ps[32 * j:32 * j + 1, 512:768],
                                     lhsT=ones_tall[32 * j:32 * j + 1, 0:1],
                                     rhs=bias_tall[32 * j:32 * j + 1, 512:768],
                                     start=False, stop=True,
                                     tile_position=(32 * j, 32 * j))
                for j in range(G if do_gic else 0):
                    if j % 2 == 0:
                        nc.scalar.copy(gic_tall[32 * j:32 * j + 1, :],
                                       gic_ps[32 * j:32 * j + 1, 0:RG])
                    else:
                        nc.vector.tensor_copy(gic_tall[32 * j:32 * j + 1, :],
                                              gic_ps[32 * j:32 * j + 1, 0:RG])

            # ---------- phase 3: gi[t] = gic + W_ih[:, E:] @ x_t ----------
            with tc.tile_pool(name="pwx", bufs=1) as pwx, \
                 tc.tile_pool(name="pwxps", bufs=2, space="PSUM") as pwxps:
                wx_sb = pwx.tile([128, EC * G * RG], BF16)
                if do_wxdma:
                    nc.sync.dma_start(wx_sb[:], wx_d[:])
                wxv = wx_sb[:].rearrange("p (c j m) -> p c j m", c=EC, j=G)
                for j in range(G if do_gi else 0):
                    rz_ps = pwxps.tile([T, 512], F32, space="PSUM", tag="girz")
                    for c in range(EC):
                        nc.tensor.matmul(rz_ps[:T, :], lhsT=dxT_bf[:, c, :],
                                         rhs=wxv[:, c, j, 0:512],
                                         start=(c == 0), stop=False)
                    nc.tensor.matmul(rz_ps[:T, :],
                                     lhsT=ones_tall[32 * j:32 * j + 1, :T],
                                     rhs=gic_tall[32 * j:32 * j + 1, 0:512],
                                     start=False, stop=True,
                                     tile_position=(32 * j, 0))
                    nc.vector.tensor_copy(girz[:, j, :], rz_ps[:T, :])

                    n_ps = pwxps.tile([T, 256], F32, space="PSUM", tag="gin")
                    for c in range(EC):
                        nc.tensor.matmul(n_ps[:T, :], lhsT=dxT_bf[:, c, :],
                                         rhs=wxv[:, c, j, 512:768],
                                         start=(c == 0), stop=False)
                    nc.tensor.matmul(n_ps[:T, :],
                                     lhsT=ones_tall[32 * j:32 * j + 1, :T],
                                     rhs=gic_tall[32 * j:32 * j + 1, 512:768],
                                     start=False, stop=True,
                                     tile_position=(32 * j, 0))
                    nbf = pwx.tile([T, 256], BF16, tag="ginbf")
                    nc.vector.tensor_copy(nbf[:], n_ps[:T, :])
                    nc.sync.dma_start(gin[j:j + 1, :], nbf[:T, :])

            # ---------- phase 4: the 65-step recurrence ----------
            whhv = whh[:].rearrange("p (c j m) -> p c j m", c=EC, j=G)
            htv4 = ht_full[:].rearrange("p (j par tt) -> p par j tt", j=4, par=2)
            with tc.tile_pool(name="rec", bufs=2) as pr, \
                 tc.tile_pool(name="recps_g", bufs=2, space="PSUM") as prg, \
                 tc.tile_pool(name="recps_s", bufs=1, space="PSUM") as prs:
                for t in range(rec_steps):
                    psg = prg.tile([128, 1024], F32, space="PSUM", tag="psg")
                    for j in range(G):
                        nc.tensor.matmul(psg[32 * j:32 * j + 1, 0:512],
                                         lhsT=ident_bf[:T, t:t + 1], rhs=girz[:T, j, :],
                                         start=True, stop=False, tile_position=(0, 32 * j))
                        for c in range(EC):
                            nc.tensor.matmul(psg[32 * j:32 * j + 1, 0:512],
                                             lhsT=h_stat[:, c:c + 1], rhs=whhv[:, c, j, 0:512],
                                             start=False, stop=(c == EC - 1),
                                             tile_position=(0, 32 * j))
                            nc.tensor.matmul(psg[32 * j:32 * j + 1, 512:768],
                                             lhsT=h_stat[:, c:c + 1], rhs=whhv[:, c, j, 512:768],
                                             start=(c == 0), stop=(c == EC - 1),
                                             tile_position=(0, 32 * j))
                    gtall = gtallA if t % 2 == 0 else gtallB
                    for j in range(G):
                        if j % 2 == 0:
                            nc.scalar.copy(gtall[32 * j:32 * j + 1, :],
                                           psg[32 * j:32 * j + 1, 0:RG])
                        else:
                            nc.vector.tensor_copy(gtall[32 * j:32 * j + 1, :],
                                                  psg[32 * j:32 * j + 1, 0:RG])
                    gcmp = prs.tile([4, 1024], F32, space="PSUM", tag="gcmp")
                    nc.tensor.matmul(gcmp[:4, 0:512], lhsT=sel4[:], rhs=gtall[:, 0:512],
                                     start=True, stop=True)
                    nc.tensor.matmul(gcmp[:4, 512:768], lhsT=sel4[:], rhs=gtall[:, 512:768],
                                     start=True, stop=True)
                    rz = pr.tile([4, 512], F32, tag="rz")
                    nc.scalar.activation(rz[:], gcmp[:4, 0:512], AF.Sigmoid)
                    t1 = pr.tile([4, 256], F32, tag="t1")
                    nc.vector.tensor_tensor(out=t1[:], in0=rz[:, 0:256],
                                            in1=gcmp[:4, 512:768], op=ALU.mult)
                    npre = pr.tile([4, 256], F32, tag="npre")
                    nc.vector.tensor_tensor(out=npre[:], in0=t1[:],
                                            in1=gin[:4, 256 * t:256 * (t + 1)], op=ALU.add)
                    n_g = pr.tile([4, 256], F32, tag="n_g")
                    nc.scalar.activation(n_g[:], npre[:], AF.Tanh)
                    d_g = pr.tile([4, 256], F32, tag="d_g")
                    nc.vector.tensor_tensor(out=d_g[:], in0=h_rows[:], in1=n_g[:],
                                            op=ALU.subtract)
                    zd = pr.tile([4, 256], F32, tag="zd")
                    nc.vector.tensor_tensor(out=zd[:], in0=rz[:, 256:512], in1=d_g[:],
                                            op=ALU.mult)
                    nc.vector.tensor_tensor(out=h_rows[:], in0=n_g[:], in1=zd[:],
                                            op=ALU.add)
                    hT_ps = prs.tile([128, 8], F32, space="PSUM", tag="hT")
                    nc.tensor.transpose(out=hT_ps[:, 0:4], in_=h_rows[:4, 0:128],
                                        identity=ident[:4, :4])
                    nc.tensor.transpose(out=hT_ps[:, 4:8], in_=h_rows[:4, 128:256],
                                        identity=ident[:4, :4])
                    # h_stat[:, 2j+par] <- hT_ps[:, par*4+j]
                    nc.vector.tensor_copy(
                        h_stat[:].rearrange("p (j par) -> p par j", par=2),
                        hT_ps[:, :].rearrange("p (par j) -> p par j", j=4))
                    # ht_full[p, (2j+par)*T + t] = relu(hT_ps[p, par*4+j])
                    nc.scalar.activation(
                        htv4[:, :, :, t:t + 1],
                        hT_ps[:, :].rearrange("p (par j) -> p par j", j=4).unsqueeze(3),
                        AF.Relu)

            # ---------- phase 5: logits = relu(H) @ out_w^T + out_b ----------
            owtv = owt_d.rearrange("p (c v) -> p c v", c=EC)
            htv = ht_full[:].rearrange("p (c tt) -> p c tt", c=EC)
            if not do_final:
                nc.sync.dma_start(out_d[0:T, 0:T], ones_tall[:T, :T])
            with tc.tile_pool(name="fin", bufs=2) as pf, \
                 tc.tile_pool(name="finps", bufs=2, space="PSUM") as pfps:
                outb_sb = pf.tile([1, VP], F32, tag="outb")
                if do_final:
                    nc.sync.dma_start(outb_sb[:], outb_d[:])
                for vb in range(VP // 512 if do_final else 0):
                    wchunk = pf.tile([128, EC, 512], BF16, tag="wchunk")
                    nc.sync.dma_start(wchunk[:], owtv[:, :, 512 * vb:512 * (vb + 1)])
                    ops = pfps.tile([T, 512], F32, space="PSUM", tag="ops")
                    for c in range(EC):
                        nc.tensor.matmul(ops[:T, :], lhsT=htv[:, c, :],
                                         rhs=wchunk[:, c, :],
                                         start=(c == 0), stop=False)
                    nc.tensor.matmul(ops[:T, :], lhsT=ones_tall[:1, :T],
                                     rhs=outb_sb[:1, 512 * vb:512 * (vb + 1)],
                                     start=False, stop=True)
                    osb = pf.tile([T, 512], F32, tag="osb")
                    nc.vector.tensor_copy(osb[:], ops[:T, :])
                    nc.sync.dma_start(out_d[:, 512 * vb:512 * (vb + 1)], osb[:])

    nc.compile()
    return nc


def _prep_inputs(inp):
    idx_enc = np.concatenate([inp["input_diagnosis"], inp["input_procedure"],
                              inp["input_medicine"]]).astype(np.int64)
    tokens = np.concatenate([np.array([V0], np.int64),
                             inp["dec_tokens"].astype(np.int64)])
    enc_emb = np.asarray(inp["enc_emb"], np.float32)
    dec_emb = np.asarray(inp["dec_emb"], np.float32)

    ctx = np.ascontiguousarray(enc_emb[idx_enc])                       # [320, 1024]
    decx = np.ascontiguousarray(dec_emb[tokens])                       # [65, 1024]
    we = np.ascontiguousarray(np.asarray(inp["attn_w"], np.float32)[0, E:]).reshape(1, E)

    w_ih = np.asarray(inp["gru_w_ih"], np.float32)                     # [3072, 2048]
    w_hh = np.asarray(inp["gru_w_hh"], np.float32)                     # [3072, 1024]
    b_ih = np.asarray(inp["gru_b_ih"], np.float32)
    b_hh = np.asarray(inp["gru_b_hh"], np.float32)
    assert not np.any(b_hh[2 * E:]), "nonzero b_hh n-gate not supported on device"

    whh_arr = _arrange_w(w_hh)                                         # [128, 24576] f32
    wc_arr = _arrange_w(np.ascontiguousarray(w_ih[:, :E])).astype(NP_BF16)
    wx_arr = _arrange_w(np.ascontiguousarray(w_ih[:, E:])).astype(NP_BF16)
    bias = b_ih.copy()
    bias[:2 * E] += b_hh[:2 * E]
    bias_arr = _bias_tall(bias)                                        # [128, 768] f32

    out_w = np.asarray(inp["out_w"], np.float32)
    out_b = np.asarray(inp["out_b"], np.float32)
    owp = np.zeros((NCORES * VP, E), np.float32)
    owp[:V] = out_w
    obp = np.zeros(NCORES * VP, np.float32)
    obp[:V] = out_b

    base = {"ctx": ctx, "decx": decx, "we": we, "whh": whh_arr,
            "wc": wc_arr, "wx": wx_arr, "bias": bias_arr}
    in_maps = []
    for i in range(NCORES):
        s = owp[i * VP:(i + 1) * VP]                                   # [4096, 1024]
        owt = np.ascontiguousarray(
            s.reshape(VP, EC, 128).transpose(2, 1, 0)).astype(NP_BF16).reshape(128, EC * VP)
        m = dict(base)
        m["owt"] = owt
        m["outb"] = np.ascontiguousarray(obp[i * VP:(i + 1) * VP]).reshape(1, VP)
        in_maps.append(m)
    return in_maps


def kernel(**inputs):
    if "nc" not in _CACHE:
        _CACHE["nc"] = build_program()
    nc = _CACHE["nc"]
    in_maps = _prep_inputs({k: np.asarray(v) for k, v in inputs.items()})
    res = run_bass_kernel_spmd(nc, in_maps, core_ids=list(range(NCORES)))
    slices = [res.results[i]["out"] for i in range(NCORES)]            # each [65, 4096]
    logits = np.concatenate(slices, axis=1)[:, :V]
    return np.ascontiguousarray(logits.astype(np.float32))



# revision 18
# speedup vs baseline: 4.5327x; 4.5327x over previous
"""Trainium2 Bass kernel for nn_LEAP_74371653697613 (GRU decoder w/ additive attention).

Structure exploited:
  - softmax(ctx_score + h.w_h + b) == softmax(ctx_score): attention weights are
    constant across decode steps -> context vector c computed once on device.
  - gi_t = W_ih @ [c; x_t] + b_ih is teacher-forced -> batched matmuls, precomputed.
  - logits don't feed back -> one big relu(H) @ out_w^T matmul at the end,
    vocab-sharded across the 8 cores (each core gets a 4096-row slice of out_w).
  - only W_hh @ h_t + gates is sequential (65 steps); it runs identically on all
    8 cores (replicated -> zero cross-core communication).

Per-step device schedule (v2):
  - 4 col-tiled groups (tile_position=(0,32j)) of bf16 matvec MMs, emitted
    c-outer/j-inner so the four 32-col strips of the PE array run concurrently;
    gi_rz[t] folded into the same PSUM accumulation via a one-hot K=65 matmul
    issued one step ahead (hides in the tail of step t-1).
  - gate math runs on FULL 128-partition tiles straight out of PSUM (rows other
    than {0,32,64,96} compute harmless garbage; engines cost by elems-per-lane,
    so the wide op is the same price and needs no bridge/compaction).
  - h' is transposed with two full 128x128 PE transposes; h_stat [128,8] is a
    single stride-32 free-dim gather from the transpose PSUM; relu(h') lands in
    ht_full off the critical path.
  - out_w slice (bf16) is prefetched into SBUF during the recurrence, so the
    final logits matmul is compute-bound.
"""
import os
import sys
import numpy as np

for _p in ("/opt/trn_rl_repo", "/root/.axon_site/_ro/trn_rl_repo"):
    if os.path.isdir(_p) and _p not in sys.path:
        sys.path.insert(0, _p)

import concourse.bass as bass
import concourse.bacc as bacc
import concourse.tile as tile
import concourse.mybir as mybir
from concourse.bass_utils import run_bass_kernel_spmd
from concourse.masks import make_identity

F32 = mybir.dt.float32
BF16 = mybir.dt.bfloat16
AF = mybir.ActivationFunctionType
ALU = mybir.AluOpType
NP_BF16 = mybir.dt.np(BF16)

E = 1024          # emb dim
EC = 8            # E / 128 chunks
T = 65            # decode steps (1 SOS + 64)
L = 320           # context rows (128 + 64 + 128)
V0 = 32000
V = V0 + 2        # 32002
NCORES = 8
VP = 4096         # per-core padded vocab slice (8 * 4096 = 32768 >= 32002)
G = 4             # col-tile groups
RG = 768          # region width per group (3 gates x 256)

_CACHE = {}


def _arrange_w(w):
    """W_ih halves [3072, 1024] -> [128, 8*4*768]: out[p, ((c*4)+j)*768 + g*256+mm]
    = w[g*1024 + j*256 + mm, c*128 + p].  (c = standard 128-chunk of the input.)"""
    x = w.reshape(3, 4, 256, EC, 128)            # g, j, mm, c, p
    x = np.transpose(x, (4, 3, 1, 0, 2))         # p, c, j, g, mm
    return np.ascontiguousarray(x).reshape(128, EC * G * RG)


def _arrange_whh(w):
    """W_hh [3072, 1024] -> [128, 8*4*768] with the h-chunk dim in k-order
    (k = 4*half + jh, covering h[256*jh + 128*half + p]):
    out[p, (k*4 + j)*768 + g*256+mm] = w[g*1024 + j*256 + mm, 256*jh + 128*half + p]."""
    x = w.reshape(3, 4, 256, 4, 2, 128)          # g, j, mm, jh, half, p
    x = np.transpose(x, (5, 4, 3, 1, 0, 2))      # p, half, jh, j, g, mm
    return np.ascontiguousarray(x).reshape(128, EC * G * RG)


def _bias_tall(b_rzn):
    """[3072] bias in gate order -> [128, 768] with row 32j = region j."""
    x = b_rzn.reshape(3, 4, 256)                 # g, j, mm
    x = np.transpose(x, (1, 0, 2)).reshape(4, RG)  # j, (g mm)
    out = np.zeros((128, RG), np.float32)
    out[::32, :] = x
    return out


def build_program():
    nc = bacc.Bacc("TRN2", target_bir_lowering=False, debug=False, num_devices=NCORES)

    ctx_d = nc.dram_tensor("ctx", [L, E], F32, kind="ExternalInput").ap()
    decx_d = nc.dram_tensor("decx", [T, E], F32, kind="ExternalInput").ap()
    we_d = nc.dram_tensor("we", [1, E], F32, kind="ExternalInput").ap()
    whh_d = nc.dram_tensor("whh", [128, EC * G * RG], BF16, kind="ExternalInput").ap()
    wc_d = nc.dram_tensor("wc", [128, EC * G * RG], BF16, kind="ExternalInput").ap()
    wx_d = nc.dram_tensor("wx", [128, EC * G * RG], BF16, kind="ExternalInput").ap()
    bias_d = nc.dram_tensor("bias", [128, RG], F32, kind="ExternalInput").ap()
    h0k_d = nc.dram_tensor("h0k", [128, EC], BF16, kind="ExternalInput").ap()
    owt_d = nc.dram_tensor("owt", [128, EC * VP], BF16, kind="ExternalInput").ap()
    outb_d = nc.dram_tensor("outb", [1, VP], BF16, kind="ExternalInput").ap()
    out_d = nc.dram_tensor("out", [T, VP], F32, kind="ExternalOutput").ap()

    with tile.TileContext(nc) as tc:
        with tc.tile_pool(name="persist", bufs=1) as pp:
            # ---------- persistent constants ----------
            whh = pp.tile([128, EC * G * RG], BF16)
            nc.sync.dma_start(whh[:], whh_d[:])

            ident = pp.tile([128, 128], F32)
            make_identity(nc, ident[:])
            ident_bf = pp.tile([128, 128], BF16)
            nc.vector.tensor_copy(ident_bf[:], ident[:])

            ones_tall = pp.tile([128, T], F32)
            nc.gpsimd.memset(ones_tall[:], 1.0)
            ones_col = pp.tile([128, 1], F32)
            nc.gpsimd.memset(ones_col[:], 1.0)
            ones_row = pp.tile([1, 128], F32)
            nc.gpsimd.memset(ones_row[:], 1.0)

            bias_tall = pp.tile([128, RG], F32)    # (b_ih + b_hh) rz | b_ih n, rows at 32j
            nc.sync.dma_start(bias_tall[:], bias_d[:])

            girz = pp.tile([T, G, 512], BF16)      # gi rz-part, partition = t
            gin_tall = pp.tile([128, T * 256], BF16)  # gi n-part at rows 32j
            gic_tall = pp.tile([128, RG], F32)     # const part of gi, rows at 32j
            ht_full = pp.tile([128, EC * T], BF16)  # [p, k, t] = relu(h_t[chunk k])
            h_stat = pp.tile([128, EC], BF16)      # h in k-order columns (MM stationary)
            h_tall = pp.tile([128, 256], F32)      # wide h: row 32j, col m = h[256j+m]
            cT_bf = pp.tile([128, EC], BF16)
            dxT_bf = pp.tile([128, EC, T], BF16)

            nc.gpsimd.memset(h_tall[:], 0.0)
            nc.gpsimd.memset(gin_tall[:], 0.0)
            for j in range(G):
                nc.sync.dma_start(h_tall[32 * j:32 * j + 1, :],
                                  decx_d[0:1, 256 * j:256 * (j + 1)])
            # h0 in k-order: h_stat[p, half*4+jh] = decx[0, 256*jh+128*half+p]
            nc.sync.dma_start(h_stat[:], h0k_d[:])

            # ---------- phase 1: attention (constant across steps) ----------
            with tc.tile_pool(name="ph1", bufs=1) as p1, \
                 tc.tile_pool(name="ph1ps", bufs=1, space="PSUM") as p1ps:
                we_sb = p1.tile([1, E], F32)
                nc.sync.dma_start(we_sb[:], we_d[:])
                rows3 = (128, 128, 64)
                ctx_sb = []
                for i, rows in enumerate(rows3):
                    t_ = p1.tile([128, E], F32, tag=f"ctx{i}")
                    nc.sync.dma_start(t_[:rows, :], ctx_d[128 * i:128 * i + rows, :])
                    ctx_sb.append(t_)
                decx_sb = p1.tile([T, E], F32)
                nc.sync.dma_start(decx_sb[:], decx_d[:])

                # replicate w_e across partitions via K=1 matmul
                werep_ps = p1ps.tile([128, E], F32, space="PSUM")
                for half in range(2):
                    nc.tensor.matmul(werep_ps[:, 512 * half:512 * (half + 1)],
                                     lhsT=ones_row[:1, :],
                                     rhs=we_sb[:1, 512 * half:512 * (half + 1)],
                                     start=True, stop=True)
                werep = p1.tile([128, E], F32)
                nc.vector.tensor_copy(werep[:], werep_ps[:])

                scratch = p1.tile([128, E], F32)
                escore = [p1.tile([128, 1], F32, tag=f"esc{i}", name=f"esc{i}")
                          for i in range(3)]
                for i, rows in enumerate(rows3):
                    sc = p1.tile([128, 1], F32, tag=f"sc{i}")
                    nc.vector.tensor_tensor(out=scratch[:rows, :],
                                            in0=ctx_sb[i][:rows, :],
                                            in1=werep[:rows, :], op=ALU.mult)
                    nc.vector.tensor_reduce(out=sc[:rows, :], in_=scratch[:rows, :],
                                            axis=mybir.AxisListType.X, op=ALU.add)
                    nc.scalar.activation(escore[i][:rows, :], sc[:rows, :], AF.Exp)
                ssum_ps = p1ps.tile([1, 1], F32, space="PSUM")
                for i, rows in enumerate(rows3):
                    nc.tensor.matmul(ssum_ps[:1, :1], lhsT=escore[i][:rows, :1],
                                     rhs=ones_col[:rows, :1],
                                     start=(i == 0), stop=(i == 2))
                rsum = p1.tile([1, 1], F32)
                nc.vector.reciprocal(rsum[:], ssum_ps[:1, :1])

                cun_ps = p1ps.tile([1, E], F32, space="PSUM")
                for half in range(2):
                    for i, rows in enumerate(rows3):
                        nc.tensor.matmul(cun_ps[:1, 512 * half:512 * (half + 1)],
                                         lhsT=escore[i][:rows, :1],
                                         rhs=ctx_sb[i][:rows, 512 * half:512 * (half + 1)],
                                         start=(i == 0), stop=(i == 2))
                c_sb = p1.tile([1, E], F32)
                nc.vector.tensor_scalar_mul(c_sb[:], cun_ps[:1, :], rsum[:1, :1])

                # c^T [128, 8] bf16 via PE transposes
                cT_ps = p1ps.tile([128, EC], F32, space="PSUM")
                for k in range(EC):
                    nc.tensor.transpose(out=cT_ps[:, k:k + 1],
                                        in_=c_sb[:1, 128 * k:128 * (k + 1)],
                                        identity=ident[:1, :1])
                nc.vector.tensor_copy(cT_bf[:], cT_ps[:])

                # dec_x^T [128, 8, 65] bf16 via PE transposes
                dxT_ps = p1ps.tile([128, T], F32, space="PSUM")
                for k in range(EC):
                    nc.tensor.transpose(out=dxT_ps[:, :],
                                        in_=decx_sb[:T, 128 * k:128 * (k + 1)],
                                        identity=ident[:T, :T])
                    nc.vector.tensor_copy(dxT_bf[:, k, :], dxT_ps[:, :])

            # ---------- phase 2: gic = W_ih[:, :E] @ c + biases ----------
            with tc.tile_pool(name="pwc", bufs=1) as pwc, \
                 tc.tile_pool(name="pwcps", bufs=1, space="PSUM") as pwcps:
                wc_sb = pwc.tile([128, EC * G * RG], BF16)
                nc.sync.dma_start(wc_sb[:], wc_d[:])
                wcv = wc_sb[:].rearrange("p (c j m) -> p c j m", c=EC, j=G)
                gic_ps = pwcps.tile([128, 1024], F32, space="PSUM")
                for j in range(G):
                    for c in range(EC):
                        nc.tensor.matmul(gic_ps[32 * j:32 * j + 1, 0:512],
                                         lhsT=cT_bf[:, c:c + 1], rhs=wcv[:, c, j, 0:512],
                                         start=(c == 0), stop=False,
                                         tile_position=(0, 32 * j))
                        nc.tensor.matmul(gic_ps[32 * j:32 * j + 1, 512:768],
                                         lhsT=cT_bf[:, c:c + 1], rhs=wcv[:, c, j, 512:768],
                                         start=(c == 0), stop=False,
                                         tile_position=(0, 32 * j))
                    # + (b_ih + b_hh)_rz | b_ih_n  via K=1 matmul
                    nc.tensor.matmul(gic_ps[32 * j:32 * j + 1, 0:512],
                                     lhsT=ones_tall[32 * j:32 * j + 1, 0:1],
                                     rhs=bias_tall[32 * j:32 * j + 1, 0:512],
                                     start=False, stop=True,
                                     tile_position=(32 * j, 32 * j))
                    nc.tensor.matmul(gic_ps[32 * j:32 * j + 1, 512:768],
                                     lhsT=ones_tall[32 * j:32 * j + 1, 0:1],
                                     rhs=bias_tall[32 * j:32 * j + 1, 512:768],
                                     start=False, stop=True,
                                     tile_position=(32 * j, 32 * j))
                for j in range(G):
                    if j % 2 == 0:
                        nc.scalar.copy(gic_tall[32 * j:32 * j + 1, :],
                                       gic_ps[32 * j:32 * j + 1, 0:RG])
                    else:
                        nc.vector.tensor_copy(gic_tall[32 * j:32 * j + 1, :],
                                              gic_ps[32 * j:32 * j + 1, 0:RG])

            # ---------- phase 3: gi[t] = gic + W_ih[:, E:] @ x_t ----------
            with tc.tile_pool(name="pwx", bufs=1) as pwx, \
                 tc.tile_pool(name="pwxps", bufs=2, space="PSUM") as pwxps:
                wx_sb = pwx.tile([128, EC * G * RG], BF16)
                nc.sync.dma_start(wx_sb[:], wx_d[:])
                wxv = wx_sb[:].rearrange("p (c j m) -> p c j m", c=EC, j=G)
                for j in range(G):
                    rz_ps = pwxps.tile([T, 512], F32, space="PSUM", tag="girz")
                    for c in range(EC):
                        nc.tensor.matmul(rz_ps[:T, :], lhsT=dxT_bf[:, c, :],
                                         rhs=wxv[:, c, j, 0:512],
                                         start=(c == 0), stop=False)
                    nc.tensor.matmul(rz_ps[:T, :],
                                     lhsT=ones_tall[32 * j:32 * j + 1, :T],
                                     rhs=gic_tall[32 * j:32 * j + 1, 0:512],
                                     start=False, stop=True,
                                     tile_position=(32 * j, 0))
                    nc.vector.tensor_copy(girz[:, j, :], rz_ps[:T, :])

                    n_ps = pwxps.tile([T, 256], F32, space="PSUM", tag="gin")
                    for c in range(EC):
                        nc.tensor.matmul(n_ps[:T, :], lhsT=dxT_bf[:, c, :],
                                         rhs=wxv[:, c, j, 512:768],
                                         start=(c == 0), stop=False)
                    nc.tensor.matmul(n_ps[:T, :],
                                     lhsT=ones_tall[32 * j:32 * j + 1, :T],
                                     rhs=gic_tall[32 * j:32 * j + 1, 512:768],
                                     start=False, stop=True,
                                     tile_position=(32 * j, 0))
                    nbf = pwx.tile([T, 256], BF16, tag="ginbf")
                    nc.vector.tensor_copy(nbf[:], n_ps[:T, :])
                    nc.sync.dma_start(gin_tall[32 * j:32 * j + 1, :], nbf[:T, :])

            # ---------- phase 4: the 65-step recurrence ----------
            whhv = whh[:].rearrange("p (k j m) -> p k j m", k=EC, j=G)
            ginv = gin_tall[:].rearrange("p (t m) -> p t m", t=T)
            htv = ht_full[:].rearrange("p (k t) -> p k t", k=EC)
            with tc.tile_pool(name="recsb", bufs=1) as prl, \
                 tc.tile_pool(name="rec", bufs=2) as pr, \
                 tc.tile_pool(name="recps_g", bufs=1, space="PSUM") as prg, \
                 tc.tile_pool(name="recps_t", bufs=2, space="PSUM") as prs:
                # prefetch out_w^T slice during the recurrence (queued after wx)
                owt_sb = prl.tile([128, EC * VP], BF16)
                nc.sync.dma_start(owt_sb[:], owt_d[:])
                outb_sb = prl.tile([1, VP], BF16)
                nc.sync.dma_start(outb_sb[:], outb_d[:])
                ones_bf = prl.tile([1, T], BF16)
                nc.vector.tensor_copy(ones_bf[:], ones_tall[0:1, :])
                psgs = [prg.tile([128, 768], F32, space="PSUM", tag=f"psg{t % 2}",
                                 name=f"psg{t % 2}")
                        for t in range(2)]
                # pre-fold gi_rz[0] into psum 0
                for j in range(G):
                    nc.tensor.matmul(psgs[0][32 * j:32 * j + 1, 0:512],
                                     lhsT=ident_bf[:T, 0:1], rhs=girz[:T, j, :],
                                     start=True, stop=False, tile_position=(0, 32 * j))
                for t in range(T):
                    psg = psgs[t % 2]
                    # rz matvec: c-outer / j-inner so the 4 col strips overlap
                    for c in range(EC):
                        for j in range(G):
                            nc.tensor.matmul(psg[32 * j:32 * j + 1, 0:512],
                                             lhsT=h_stat[:, c:c + 1],
                                             rhs=whhv[:, c, j, 0:512],
                                             start=False, stop=(c == EC - 1),
                                             tile_position=(0, 32 * j))
                    # n matvec
                    for c in range(EC):
                        for j in range(G):
                            nc.tensor.matmul(psg[32 * j:32 * j + 1, 512:768],
                                             lhsT=h_stat[:, c:c + 1],
                                             rhs=whhv[:, c, j, 512:768],
                                             start=(c == 0), stop=(c == EC - 1),
                                             tile_position=(0, 32 * j))
                    # pre-fold gi_rz[t+1] into the alternate psum (runs in tail idle)
                    if t + 1 < T:
                        nxt = psgs[(t + 1) % 2]
                        for j in range(G):
                            nc.tensor.matmul(nxt[32 * j:32 * j + 1, 0:512],
                                             lhsT=ident_bf[:T, t + 1:t + 2],
                                             rhs=girz[:T, j, :],
                                             start=True, stop=False,
                                             tile_position=(0, 32 * j))

                    # gate tail on full-width tiles (junk rows are harmless)
                    rz_sb = pr.tile([128, 512], BF16, tag="rz")
                    nc.scalar.activation(rz_sb[:], psg[:, 0:512], AF.Sigmoid)
                    t1 = pr.tile([128, 256], BF16, tag="t1")
                    nc.vector.tensor_tensor(out=t1[:], in0=rz_sb[:, 0:256],
                                            in1=psg[:, 512:768], op=ALU.mult)
                    npre = pr.tile([128, 256], BF16, tag="npre")
                    nc.vector.tensor_tensor(out=npre[:], in0=t1[:],
                                            in1=ginv[:, t, :], op=ALU.add)
                    n_sb = pr.tile([128, 256], F32, tag="n_g")
                    nc.scalar.activation(n_sb[:], npre[:], AF.Tanh)
                    d_g = pr.tile([128, 256], F32, tag="d_g")
                    nc.vector.tensor_tensor(out=d_g[:], in0=h_tall[:], in1=n_sb[:],
                                            op=ALU.subtract)
                    zd = pr.tile([128, 256], F32, tag="zd")
                    nc.vector.tensor_tensor(out=zd[:], in0=rz_sb[:, 256:512],
                                            in1=d_g[:], op=ALU.mult)
                    nc.vector.tensor_tensor(out=h_tall[:], in0=n_sb[:], in1=zd[:],
                                            op=ALU.add)
                    # h' -> [128, 8] k-order via 2 full transposes + strided gather
                    tp = prs.tile([128, 256], F32, space="PSUM", tag="tp")
                    nc.tensor.transpose(out=tp[:, 0:128], in_=h_tall[:, 0:128],
                                        identity=ident[:, :])
                    nc.tensor.transpose(out=tp[:, 128:256], in_=h_tall[:, 128:256],
                                        identity=ident[:, :])
                    tpv = tp[:].rearrange("p (k s) -> p k s", k=EC)
                    nc.vector.tensor_copy(h_stat[:], tpv[:, :, 0])
                    nc.scalar.activation(htv[:, :, t:t + 1],
                                         tpv[:, :, 0:1], AF.Relu)

                # ------ phase 5: logits = relu(H) @ out_w^T + out_b ------
                owtv = owt_sb[:].rearrange("p (k v) -> p k v", k=EC)
                with tc.tile_pool(name="fin", bufs=2) as pf, \
                     tc.tile_pool(name="finps", bufs=2, space="PSUM") as pfps:
                    for vb in range(VP // 512):
                        ops = pfps.tile([T, 512], F32, space="PSUM", tag="ops")
                        for k in range(EC):
                            nc.tensor.matmul(ops[:T, :], lhsT=htv[:, k, :],
                                             rhs=owtv[:, k, 512 * vb:512 * (vb + 1)],
                                             start=(k == 0), stop=False)
                        nc.tensor.matmul(ops[:T, :], lhsT=ones_bf[:1, :T],
                                         rhs=outb_sb[:1, 512 * vb:512 * (vb + 1)],
                                         start=False, stop=True)
                        osb = pf.tile([T, 512], F32, tag="osb")
                        nc.vector.tensor_copy(osb[:], ops[:T, :])
                        nc.sync.dma_start(out_d[:, 512 * vb:512 * (vb + 1)], osb[:])

    nc.compile()
    return nc


def _prep_inputs(inp):
    idx_enc = np.concatenate([inp["input_diagnosis"], inp["input_procedure"],
                              inp["input_medicine"]]).astype(np.int64)
    tokens = np.concatenate([np.array([V0], np.int64),
                             inp["dec_tokens"].astype(np.int64)])
    enc_emb = np.asarray(inp["enc_emb"], np.float32)
    dec_emb = np.asarray(inp["dec_emb"], np.float32)

    ctx = np.ascontiguousarray(enc_emb[idx_enc])                       # [320, 1024]
    decx = np.ascontiguousarray(dec_emb[tokens])                       # [65, 1024]
    we = np.ascontiguousarray(np.asarray(inp["attn_w"], np.float32)[0, E:]).reshape(1, E)

    w_ih = np.asarray(inp["gru_w_ih"], np.float32)                     # [3072, 2048]
    w_hh = np.asarray(inp["gru_w_hh"], np.float32)                     # [3072, 1024]
    b_ih = np.asarray(inp["gru_b_ih"], np.float32)
    b_hh = np.asarray(inp["gru_b_hh"], np.float32)
    assert not np.any(b_hh[2 * E:]), "nonzero b_hh n-gate not supported on device"

    whh_arr = _arrange_whh(w_hh).astype(NP_BF16)                       # [128, 24576]
    wc_arr = _arrange_w(np.ascontiguousarray(w_ih[:, :E])).astype(NP_BF16)
    wx_arr = _arrange_w(np.ascontiguousarray(w_ih[:, E:])).astype(NP_BF16)
    bias = b_ih.copy()
    bias[:2 * E] += b_hh[:2 * E]
    bias_arr = _bias_tall(bias)                                        # [128, 768] f32

    out_w = np.asarray(inp["out_w"], np.float32)
    out_b = np.asarray(inp["out_b"], np.float32)
    owp = np.zeros((NCORES * VP, E), np.float32)
    owp[:V] = out_w
    obp = np.zeros(NCORES * VP, np.float32)
    obp[:V] = out_b

    # h0 in k-order columns: h0k[p, 4*half + jh] = decx[0, 256*jh + 128*half + p]
    h0k = np.ascontiguousarray(
        np.transpose(decx[0].reshape(4, 2, 128), (2, 1, 0))).reshape(128, EC).astype(NP_BF16)

    base = {"ctx": ctx, "decx": decx, "we": we, "whh": whh_arr,
            "wc": wc_arr, "wx": wx_arr, "bias": bias_arr, "h0k": h0k}
    in_maps = []
    for i in range(NCORES):
        s = owp[i * VP:(i + 1) * VP]                                   # [4096, 1024]
        # owt[p, k*VP + v] = s[v, 256*(k%4) + 128*(k//4) + p]
        x = s.reshape(VP, 4, 2, 128)                                   # v, jh, half, p
        owt = np.ascontiguousarray(
            np.transpose(x, (3, 2, 1, 0))).astype(NP_BF16).reshape(128, EC * VP)
        m = dict(base)
        m["owt"] = owt
        m["outb"] = np.ascontiguousarray(
            obp[i * VP:(i + 1) * VP]).reshape(1, VP).astype(NP_BF16)
        in_maps.append(m)
    return in_maps


def kernel(**inputs):
    if "nc" not in _CACHE:
        _CACHE["nc"] = build_program()
    nc = _CACHE["nc"]
    in_maps = _prep_inputs({k: np.asarray(v) for k, v in inputs.items()})
    res = run_bass_kernel_spmd(nc, in_maps, core_ids=list(range(NCORES)))
    slices = [res.results[i]["out"] for i in range(NCORES)]            # each [65, 4096]
    logits = np.concatenate(slices, axis=1)[:, :V]
    return np.ascontiguousarray(logits.astype(np.float32))


# revision 27
# speedup vs baseline: 5.9838x; 1.3201x over previous
"""Trainium2 Bass kernel for nn_LEAP_74371653697613 (GRU decoder w/ additive attention).

Structure exploited:
  - softmax(ctx_score + h.w_h + b) == softmax(ctx_score): attention weights are
    constant across decode steps -> context vector c computed once on device.
  - gi_t = W_ih @ [c; x_t] + b_ih is teacher-forced -> batched matmuls, precomputed.
  - logits don't feed back -> one big relu(H) @ out_w^T matmul at the end,
    vocab-sharded across the 8 cores (each core gets a 4096-row slice of out_w).
  - only W_hh @ h_t + gates is sequential (65 steps); it runs identically on all
    8 cores (replicated -> zero cross-core communication).

Per-step device schedule (v2):
  - 4 col-tiled groups (tile_position=(0,32j)) of bf16 matvec MMs, emitted
    c-outer/j-inner so the four 32-col strips of the PE array run concurrently;
    gi_rz[t] folded into the same PSUM accumulation via a one-hot K=65 matmul
    issued one step ahead (hides in the tail of step t-1).
  - gate math runs on FULL 128-partition tiles straight out of PSUM (rows other
    than {0,32,64,96} compute harmless garbage; engines cost by elems-per-lane,
    so the wide op is the same price and needs no bridge/compaction).
  - h' is transposed with two full 128x128 PE transposes; h_stat [128,8] is a
    single stride-32 free-dim gather from the transpose PSUM; relu(h') lands in
    ht_full off the critical path.
  - out_w slice (bf16) is prefetched into SBUF during the recurrence, so the
    final logits matmul is compute-bound.
"""
import os
import sys
import numpy as np

for _p in ("/opt/trn_rl_repo", "/root/.axon_site/_ro/trn_rl_repo"):
    if os.path.isdir(_p) and _p not in sys.path:
        sys.path.insert(0, _p)

import concourse.bass as bass
import concourse.bacc as bacc
import concourse.tile as tile
import concourse.mybir as mybir
from concourse.bass_utils import run_bass_kernel_spmd
from concourse.masks import make_identity

F32 = mybir.dt.float32
BF16 = mybir.dt.bfloat16
AF = mybir.ActivationFunctionType
ALU = mybir.AluOpType
NP_BF16 = mybir.dt.np(BF16)

E = 1024          # emb dim
EC = 8            # E / 128 chunks
T = 65            # decode steps (1 SOS + 64)
L = 320           # context rows (128 + 64 + 128)
V0 = 32000
V = V0 + 2        # 32002
NCORES = 8
VP = 4096         # per-core padded vocab slice (8 * 4096 = 32768 >= 32002)
G = 4             # col-tile groups
RG = 768          # region width per group (3 gates x 256)

_CACHE = {}


def _arrange_w(w):
    """W_ih halves [3072, 1024] -> [128, 8*4*768]: out[p, ((c*4)+j)*768 + g*256+mm]
    = w[g*1024 + j*256 + mm, c*128 + p].  (c = standard 128-chunk of the input.)"""
    x = w.reshape(3, 4, 256, EC, 128)            # g, j, mm, c, p
    x = np.transpose(x, (4, 3, 1, 0, 2))         # p, c, j, g, mm
    return np.ascontiguousarray(x).reshape(128, EC * G * RG)


def _arrange_whh(w):
    """W_hh [3072, 1024] -> [128, 8*4*768] with the h-chunk dim in k-order
    (k = 4*half + jh, covering h[256*jh + 128*half + p]):
    out[p, (k*4 + j)*768 + g*256+mm] = w[g*1024 + j*256 + mm, 256*jh + 128*half + p]."""
    x = w.reshape(3, 4, 256, 4, 2, 128)          # g, j, mm, jh, half, p
    x = np.transpose(x, (5, 4, 3, 1, 0, 2))      # p, half, jh, j, g, mm
    return np.ascontiguousarray(x).reshape(128, EC * G * RG)


def _bias_tall(b_rzn):
    """[3072] bias in gate order -> [128, 768] with row 32j = region j."""
    x = b_rzn.reshape(3, 4, 256)                 # g, j, mm
    x = np.transpose(x, (1, 0, 2)).reshape(4, RG)  # j, (g mm)
    out = np.zeros((128, RG), np.float32)
    out[::32, :] = x
    return out


def build_program():
    nc = bacc.Bacc("TRN2", target_bir_lowering=False, debug=False, num_devices=NCORES)

    ctx_d = nc.dram_tensor("ctx", [L, E], F32, kind="ExternalInput").ap()
    decx_d = nc.dram_tensor("decx", [T, E], F32, kind="ExternalInput").ap()
    we_d = nc.dram_tensor("we", [1, E], F32, kind="ExternalInput").ap()
    whh_d = nc.dram_tensor("whh", [128, EC * G * RG], BF16, kind="ExternalInput").ap()
    wc_d = nc.dram_tensor("wc", [128, EC * G * RG], BF16, kind="ExternalInput").ap()
    wx_d = nc.dram_tensor("wx", [128, EC * G * RG], BF16, kind="ExternalInput").ap()
    bias_d = nc.dram_tensor("bias", [128, RG], F32, kind="ExternalInput").ap()
    h0k_d = nc.dram_tensor("h0k", [128, EC], BF16, kind="ExternalInput").ap()
    owt_d = nc.dram_tensor("owt", [128, EC * VP], BF16, kind="ExternalInput").ap()
    outb_d = nc.dram_tensor("outb", [1, VP], BF16, kind="ExternalInput").ap()
    out_d = nc.dram_tensor("out", [T, VP], F32, kind="ExternalOutput").ap()

    with tile.TileContext(nc) as tc:
        with tc.tile_pool(name="persist", bufs=1) as pp:
            # ---------- persistent constants ----------
            # (whh DMA is emitted after wc/wx so the small phase-1 loads and
            # the phase-2/3 weights win the DMA queues first)
            whh = pp.tile([128, EC * G * RG], BF16)

            ident = pp.tile([128, 128], F32)
            make_identity(nc, ident[:])
            ident_bf = pp.tile([128, 128], BF16)
            nc.vector.tensor_copy(ident_bf[:], ident[:])

            ones_tall = pp.tile([128, T], F32)
            nc.gpsimd.memset(ones_tall[:], 1.0)
            ones_col = pp.tile([128, 1], F32)
            nc.gpsimd.memset(ones_col[:], 1.0)
            ones_row = pp.tile([1, 128], F32)
            nc.gpsimd.memset(ones_row[:], 1.0)

            bias_tall = pp.tile([128, RG], F32)    # (b_ih + b_hh) rz | b_ih n, rows at 32j
            nc.sync.dma_start(bias_tall[:], bias_d[:])

            girz = pp.tile([T, G, 512], BF16)      # gi rz-part, partition = t
            gin_tall = pp.tile([128, T * 256], BF16)  # gi n-part at rows 32j
            gic_tall = pp.tile([128, RG], F32)     # const part of gi, rows at 32j
            ht_full = pp.tile([128, EC * T], BF16)  # [p, k, t] = relu(h_t[chunk k])
            h_stat = pp.tile([128, EC], BF16)      # h in k-order columns (MM stationary)
            cT_bf = pp.tile([128, EC], BF16)
            dxT_bf = pp.tile([128, EC, T], BF16)

            nc.gpsimd.memset(gin_tall[:], 0.0)
            # h0 in k-order: h_stat[p, half*4+jh] = decx[0, 256*jh+128*half+p]
            nc.sync.dma_start(h_stat[:], h0k_d[:])

            # ---------- phase 1: attention (constant across steps) ----------
            with tc.tile_pool(name="ph1", bufs=1) as p1, \
                 tc.tile_pool(name="ph1ps", bufs=1, space="PSUM") as p1ps:
                # dec_x first: phase 3 only needs dxT, so get it ready early
                decx_sb = p1.tile([T, E], F32)
                nc.sync.dma_start(decx_sb[:], decx_d[:])
                we_sb = p1.tile([1, E], F32)
                nc.sync.dma_start(we_sb[:], we_d[:])
                rows3 = (128, 128, 64)
                ctx_sb = []
                for i, rows in enumerate(rows3):
                    t_ = p1.tile([128, E], F32, tag=f"ctx{i}")
                    nc.sync.dma_start(t_[:rows, :], ctx_d[128 * i:128 * i + rows, :])
                    ctx_sb.append(t_)

                # dec_x^T [128, 8, 65] bf16 via PE transposes (bf16 in for speed)
                decx_bf = p1.tile([T, E], BF16)
                nc.vector.tensor_copy(decx_bf[:], decx_sb[:])
                dxT_ps = p1ps.tile([128, T], BF16, space="PSUM")
                for k in range(EC):
                    nc.tensor.transpose(out=dxT_ps[:, :],
                                        in_=decx_bf[:T, 128 * k:128 * (k + 1)],
                                        identity=ident_bf[:T, :T])
                    nc.vector.tensor_copy(dxT_bf[:, k, :], dxT_ps[:, :])

                # replicate w_e across partitions via K=1 matmul
                werep_ps = p1ps.tile([128, E], F32, space="PSUM")
                for half in range(2):
                    nc.tensor.matmul(werep_ps[:, 512 * half:512 * (half + 1)],
                                     lhsT=ones_row[:1, :],
                                     rhs=we_sb[:1, 512 * half:512 * (half + 1)],
                                     start=True, stop=True)
                werep = p1.tile([128, E], F32)
                nc.vector.tensor_copy(werep[:], werep_ps[:])

                scratch = p1.tile([128, E], F32)
                escore = [p1.tile([128, 1], F32, tag=f"esc{i}", name=f"esc{i}")
                          for i in range(3)]
                for i, rows in enumerate(rows3):
                    sc = p1.tile([128, 1], F32, tag=f"sc{i}")
                    nc.vector.tensor_tensor(out=scratch[:rows, :],
                                            in0=ctx_sb[i][:rows, :],
                                            in1=werep[:rows, :], op=ALU.mult)
                    nc.vector.tensor_reduce(out=sc[:rows, :], in_=scratch[:rows, :],
                                            axis=mybir.AxisListType.X, op=ALU.add)
                    nc.scalar.activation(escore[i][:rows, :], sc[:rows, :], AF.Exp)
                ssum_ps = p1ps.tile([1, 1], F32, space="PSUM")
                for i, rows in enumerate(rows3):
                    nc.tensor.matmul(ssum_ps[:1, :1], lhsT=escore[i][:rows, :1],
                                     rhs=ones_col[:rows, :1],
                                     start=(i == 0), stop=(i == 2))
                rsum = p1.tile([1, 1], F32)
                nc.vector.reciprocal(rsum[:], ssum_ps[:1, :1])

                cun_ps = p1ps.tile([1, E], F32, space="PSUM")
                for half in range(2):
                    for i, rows in enumerate(rows3):
                        nc.tensor.matmul(cun_ps[:1, 512 * half:512 * (half + 1)],
                                         lhsT=escore[i][:rows, :1],
                                         rhs=ctx_sb[i][:rows, 512 * half:512 * (half + 1)],
                                         start=(i == 0), stop=(i == 2))
                c_sb = p1.tile([1, E], F32)
                nc.vector.tensor_scalar_mul(c_sb[:], cun_ps[:1, :], rsum[:1, :1])

                # c^T [128, 8] bf16 via PE transposes
                cT_ps = p1ps.tile([128, EC], F32, space="PSUM")
                for k in range(EC):
                    nc.tensor.transpose(out=cT_ps[:, k:k + 1],
                                        in_=c_sb[:1, 128 * k:128 * (k + 1)],
                                        identity=ident[:1, :1])
                nc.vector.tensor_copy(cT_bf[:], cT_ps[:])

            # ---------- phase 2: gic = W_ih[:, :E] @ c + biases ----------
            with tc.tile_pool(name="pwc", bufs=1) as pwc, \
                 tc.tile_pool(name="pwcps", bufs=1, space="PSUM") as pwcps:
                wc_sb = pwc.tile([128, EC * G * RG], BF16)
                nc.sync.dma_start(wc_sb[:], wc_d[:])
                wcv = wc_sb[:].rearrange("p (c j m) -> p c j m", c=EC, j=G)
                gic_ps = pwcps.tile([128, 1024], F32, space="PSUM")
                for j in range(G):
                    for c in range(EC):
                        nc.tensor.matmul(gic_ps[32 * j:32 * j + 1, 0:512],
                                         lhsT=cT_bf[:, c:c + 1], rhs=wcv[:, c, j, 0:512],
                                         start=(c == 0), stop=False,
                                         tile_position=(0, 32 * j))
                        nc.tensor.matmul(gic_ps[32 * j:32 * j + 1, 512:768],
                                         lhsT=cT_bf[:, c:c + 1], rhs=wcv[:, c, j, 512:768],
                                         start=(c == 0), stop=False,
                                         tile_position=(0, 32 * j))
                    # + (b_ih + b_hh)_rz | b_ih_n  via K=1 matmul
                    nc.tensor.matmul(gic_ps[32 * j:32 * j + 1, 0:512],
                                     lhsT=ones_tall[32 * j:32 * j + 1, 0:1],
                                     rhs=bias_tall[32 * j:32 * j + 1, 0:512],
                                     start=False, stop=True,
                                     tile_position=(32 * j, 32 * j))
                    nc.tensor.matmul(gic_ps[32 * j:32 * j + 1, 512:768],
                                     lhsT=ones_tall[32 * j:32 * j + 1, 0:1],
                                     rhs=bias_tall[32 * j:32 * j + 1, 512:768],
                                     start=False, stop=True,
                                     tile_position=(32 * j, 32 * j))
                for j in range(G):
                    if j % 2 == 0:
                        nc.scalar.copy(gic_tall[32 * j:32 * j + 1, :],
                                       gic_ps[32 * j:32 * j + 1, 0:RG])
                    else:
                        nc.vector.tensor_copy(gic_tall[32 * j:32 * j + 1, :],
                                              gic_ps[32 * j:32 * j + 1, 0:RG])

            # ---------- phase 3: gi[t] = gic + W_ih[:, E:] @ x_t ----------
            with tc.tile_pool(name="pwx", bufs=1) as pwx, \
                 tc.tile_pool(name="pwxps", bufs=2, space="PSUM") as pwxps:
                wx_sb = pwx.tile([128, EC * G * RG], BF16)
                nc.sync.dma_start(wx_sb[:], wx_d[:])
                nc.sync.dma_start(whh[:], whh_d[:])
                wxv = wx_sb[:].rearrange("p (c j m) -> p c j m", c=EC, j=G)
                for j in range(G):
                    rz_ps = pwxps.tile([T, 512], F32, space="PSUM", tag="girz")
                    for c in range(EC):
                        nc.tensor.matmul(rz_ps[:T, :], lhsT=dxT_bf[:, c, :],
                                         rhs=wxv[:, c, j, 0:512],
                                         start=(c == 0), stop=False)
                    nc.tensor.matmul(rz_ps[:T, :],
                                     lhsT=ones_tall[32 * j:32 * j + 1, :T],
                                     rhs=gic_tall[32 * j:32 * j + 1, 0:512],
                                     start=False, stop=True,
                                     tile_position=(32 * j, 0))
                    nc.vector.tensor_copy(girz[:, j, :], rz_ps[:T, :])

                    n_ps = pwxps.tile([T, 256], F32, space="PSUM", tag="gin")
                    for c in range(EC):
                        nc.tensor.matmul(n_ps[:T, :], lhsT=dxT_bf[:, c, :],
                                         rhs=wxv[:, c, j, 512:768],
                                         start=(c == 0), stop=False)
                    nc.tensor.matmul(n_ps[:T, :],
                                     lhsT=ones_tall[32 * j:32 * j + 1, :T],
                                     rhs=gic_tall[32 * j:32 * j + 1, 512:768],
                                     start=False, stop=True,
                                     tile_position=(32 * j, 0))
                    nbf = pwx.tile([T, 256], BF16, tag="ginbf")
                    nc.vector.tensor_copy(nbf[:], n_ps[:T, :])
                    nc.sync.dma_start(gin_tall[32 * j:32 * j + 1, :], nbf[:T, :])

            # ---------- phase 4: the 65-step recurrence ----------
            whhv = whh[:].rearrange("p (k j m) -> p k j m", k=EC, j=G)
            ginv = gin_tall[:].rearrange("p (t m) -> p t m", t=T)
            htv = ht_full[:].rearrange("p (k t) -> p k t", k=EC)
            with tc.tile_pool(name="recsb", bufs=1) as prl, \
                 tc.tile_pool(name="rec", bufs=2) as pr, \
                 tc.tile_pool(name="recps_g", bufs=1, space="PSUM") as prg, \
                 tc.tile_pool(name="recps_t", bufs=1, space="PSUM") as prs:
                # prefetch out_w^T slice during the recurrence (queued after wx)
                owt_sb = prl.tile([128, EC * VP], BF16)
                nc.sync.dma_start(owt_sb[:], owt_d[:])
                outb_sb = prl.tile([1, VP], BF16)
                nc.sync.dma_start(outb_sb[:], outb_d[:])
                ones_bf = prl.tile([1, T], BF16)
                nc.vector.tensor_copy(ones_bf[:], ones_tall[0:1, :])
                psgs = [prg.tile([128, 768], F32, space="PSUM", tag=f"psg{t % 2}",
                                 name=f"psg{t % 2}")
                        for t in range(2)]
                # pre-fold gi_rz[0] into psum 0
                for j in range(G):
                    nc.tensor.matmul(psgs[0][32 * j:32 * j + 1, 0:512],
                                     lhsT=ident_bf[:T, 0:1], rhs=girz[:T, j, :],
                                     start=True, stop=False, tile_position=(0, 32 * j))
                for t in range(T):
                    psg = psgs[t % 2]
                    # rz matvec: c-outer / j-inner so the 4 col strips overlap
                    for c in range(EC):
                        for j in range(G):
                            nc.tensor.matmul(psg[32 * j:32 * j + 1, 0:512],
                                             lhsT=h_stat[:, c:c + 1],
                                             rhs=whhv[:, c, j, 0:512],
                                             start=False, stop=(c == EC - 1),
                                             tile_position=(0, 32 * j))
                    # n matvec
                    for c in range(EC):
                        for j in range(G):
                            nc.tensor.matmul(psg[32 * j:32 * j + 1, 512:768],
                                             lhsT=h_stat[:, c:c + 1],
                                             rhs=whhv[:, c, j, 512:768],
                                             start=(c == 0), stop=(c == EC - 1),
                                             tile_position=(0, 32 * j))
                    # pre-fold gi_rz[t+1] into the alternate psum (runs in tail idle)
                    if t + 1 < T:
                        nxt = psgs[(t + 1) % 2]
                        for j in range(G):
                            nc.tensor.matmul(nxt[32 * j:32 * j + 1, 0:512],
                                             lhsT=ident_bf[:T, t + 1:t + 2],
                                             rhs=girz[:T, j, :],
                                             start=True, stop=False,
                                             tile_position=(0, 32 * j))

                    # gate tail on full-width tiles (junk rows are harmless).
                    # h' = z*h + (1-z)*n, computed entirely in transposed
                    # [128, 8] "k-space" so no wide h state is kept at all.
                    rz_sb = pr.tile([128, 512], BF16, tag="rz")
                    nc.scalar.activation(rz_sb[:], psg[:, 0:512], AF.Sigmoid)
                    t1 = pr.tile([128, 256], BF16, tag="t1")
                    nc.vector.tensor_tensor(out=t1[:], in0=rz_sb[:, 0:256],
                                            in1=psg[:, 512:768], op=ALU.mult)
                    npre = pr.tile([128, 256], BF16, tag="npre")
                    nc.vector.tensor_tensor(out=npre[:], in0=t1[:],
                                            in1=ginv[:, t, :], op=ALU.add)
                    # transpose z and npre to k-space (PE idles here anyway)
                    tpz = prs.tile([128, 256], BF16, space="PSUM", tag="tpz")
                    nc.tensor.transpose(out=tpz[:, 0:128], in_=rz_sb[:, 256:384],
                                        identity=ident_bf[:, :])
                    nc.tensor.transpose(out=tpz[:, 128:256], in_=rz_sb[:, 384:512],
                                        identity=ident_bf[:, :])
                    tpn = prs.tile([128, 256], BF16, space="PSUM", tag="tpn")
                    nc.tensor.transpose(out=tpn[:, 0:128], in_=npre[:, 0:128],
                                        identity=ident_bf[:, :])
                    nc.tensor.transpose(out=tpn[:, 128:256], in_=npre[:, 128:256],
                                        identity=ident_bf[:, :])
                    tpzv = tpz[:].rearrange("p (k s) -> p k s", k=EC)
                    tpnv = tpn[:].rearrange("p (k s) -> p k s", k=EC)
                    # off-chain: zh = z*h, oz = 1-z   (both [128, 8] k-space)
                    zh_k = pr.tile([128, EC], BF16, tag="zh_k")
                    nc.vector.tensor_tensor(out=zh_k[:], in0=h_stat[:],
                                            in1=tpzv[:, :, 0], op=ALU.mult)
                    oz_k = pr.tile([128, EC], BF16, tag="oz_k")
                    nc.scalar.activation(oz_k[:], tpzv[:, :, 0], AF.Identity,
                                         bias=1.0, scale=-1.0)
                    # chain: tanh -> m1 = (1-z)*n -> h' = m1 + zh
                    n_k = pr.tile([128, EC], BF16, tag="n_k")
                    nc.scalar.activation(n_k[:], tpnv[:, :, 0], AF.Tanh)
                    m1 = pr.tile([128, EC], BF16, tag="m1")
                    nc.vector.tensor_tensor(out=m1[:], in0=oz_k[:], in1=n_k[:],
                                            op=ALU.mult)
                    nc.vector.tensor_tensor(out=h_stat[:], in0=m1[:], in1=zh_k[:],
                                            op=ALU.add)
                    nc.scalar.activation(htv[:, :, t:t + 1],
                                         h_stat[:].unsqueeze(2), AF.Relu)

                # ------ phase 5: logits = relu(H) @ out_w^T + out_b ------
                owtv = owt_sb[:].rearrange("p (k v) -> p k v", k=EC)
                with tc.tile_pool(name="fin", bufs=2) as pf, \
                     tc.tile_pool(name="finps", bufs=2, space="PSUM") as pfps:
                    for vb in range(VP // 512):
                        ops = pfps.tile([T, 512], F32, space="PSUM", tag="ops")
                        for k in range(EC):
                            nc.tensor.matmul(ops[:T, :], lhsT=htv[:, k, :],
                                             rhs=owtv[:, k, 512 * vb:512 * (vb + 1)],
                                             start=(k == 0), stop=False)
                        nc.tensor.matmul(ops[:T, :], lhsT=ones_bf[:1, :T],
                                         rhs=outb_sb[:1, 512 * vb:512 * (vb + 1)],
                                         start=False, stop=True)
                        osb = pf.tile([T, 512], F32, tag="osb")
                        nc.vector.tensor_copy(osb[:], ops[:T, :])
                        nc.sync.dma_start(out_d[:, 512 * vb:512 * (vb + 1)], osb[:])

    nc.compile()
    return nc


def _prep_inputs(inp):
    idx_enc = np.concatenate([inp["input_diagnosis"], inp["input_procedure"],
                              inp["input_medicine"]]).astype(np.int64)
    tokens = np.concatenate([np.array([V0], np.int64),
                             inp["dec_tokens"].astype(np.int64)])
    enc_emb = np.asarray(inp["enc_emb"], np.float32)
    dec_emb = np.asarray(inp["dec_emb"], np.float32)

    ctx = np.ascontiguousarray(enc_emb[idx_enc])                       # [320, 1024]
    decx = np.ascontiguousarray(dec_emb[tokens])                       # [65, 1024]
    we = np.ascontiguousarray(np.asarray(inp["attn_w"], np.float32)[0, E:]).reshape(1, E)

    w_ih = np.asarray(inp["gru_w_ih"], np.float32)                     # [3072, 2048]
    w_hh = np.asarray(inp["gru_w_hh"], np.float32)                     # [3072, 1024]
    b_ih = np.asarray(inp["gru_b_ih"], np.float32)
    b_hh = np.asarray(inp["gru_b_hh"], np.float32)
    assert not np.any(b_hh[2 * E:]), "nonzero b_hh n-gate not supported on device"

    whh_arr = _arrange_whh(w_hh).astype(NP_BF16)                       # [128, 24576]
    wc_arr = _arrange_w(np.ascontiguousarray(w_ih[:, :E])).astype(NP_BF16)
    wx_arr = _arrange_w(np.ascontiguousarray(w_ih[:, E:])).astype(NP_BF16)
    bias = b_ih.copy()
    bias[:2 * E] += b_hh[:2 * E]
    bias_arr = _bias_tall(bias)                                        # [128, 768] f32

    out_w = np.asarray(inp["out_w"], np.float32)
    out_b = np.asarray(inp["out_b"], np.float32)
    owp = np.zeros((NCORES * VP, E), np.float32)
    owp[:V] = out_w
    obp = np.zeros(NCORES * VP, np.float32)
    obp[:V] = out_b

    # h0 in k-order columns: h0k[p, 4*half + jh] = decx[0, 256*jh + 128*half + p]
    h0k = np.ascontiguousarray(
        np.transpose(decx[0].reshape(4, 2, 128), (2, 1, 0))).reshape(128, EC).astype(NP_BF16)

    base = {"ctx": ctx, "decx": decx, "we": we, "whh": whh_arr,
            "wc": wc_arr, "wx": wx_arr, "bias": bias_arr, "h0k": h0k}
    in_maps = []
    for i in range(NCORES):
        s = owp[i * VP:(i + 1) * VP]                                   # [4096, 1024]
        # owt[p, k*VP + v] = s[v, 256*(k%4) + 128*(k//4) + p]
        x = s.reshape(VP, 4, 2, 128)                                   # v, jh, half, p
        owt = np.ascontiguousarray(
            np.transpose(x, (3, 2, 1, 0))).astype(NP_BF16).reshape(128, EC * VP)
        m = dict(base)
        m["owt"] = owt
        m["outb"] = np.ascontiguousarray(
            obp[i * VP:(i + 1) * VP]).reshape(1, VP).astype(NP_BF16)
        in_maps.append(m)
    return in_maps


def kernel(**inputs):
    if "nc" not in _CACHE:
        _CACHE["nc"] = build_program()
    nc = _CACHE["nc"]
    in_maps = _prep_inputs({k: np.asarray(v) for k, v in inputs.items()})
    res = run_bass_kernel_spmd(nc, in_maps, core_ids=list(range(NCORES)))
    slices = [res.results[i]["out"] for i in range(NCORES)]            # each [65, 4096]
    logits = np.concatenate(slices, axis=1)[:, :V]
    return np.ascontiguousarray(logits.astype(np.float32))


# revision 36
# speedup vs baseline: 6.1869x; 1.0339x over previous
"""Trainium2 Bass kernel for nn_LEAP_74371653697613 (GRU decoder w/ additive attention).

Structure exploited:
  - softmax(ctx_score + h.w_h + b) == softmax(ctx_score): attention weights are
    constant across decode steps -> context vector c computed once on device.
  - gi_t = W_ih @ [c; x_t] + b_ih is teacher-forced -> batched matmuls, precomputed.
  - logits don't feed back -> one big relu(H) @ out_w^T matmul at the end,
    vocab-sharded across the 8 cores (each core gets a 4096-row slice of out_w).
  - only W_hh @ h_t + gates is sequential (65 steps); it runs identically on all
    8 cores (replicated -> zero cross-core communication).

Per-step device schedule (v2):
  - 4 col-tiled groups (tile_position=(0,32j)) of bf16 matvec MMs, emitted
    c-outer/j-inner so the four 32-col strips of the PE array run concurrently;
    gi_rz[t] folded into the same PSUM accumulation via a one-hot K=65 matmul
    issued one step ahead (hides in the tail of step t-1).
  - gate math runs on FULL 128-partition tiles straight out of PSUM (rows other
    than {0,32,64,96} compute harmless garbage; engines cost by elems-per-lane,
    so the wide op is the same price and needs no bridge/compaction).
  - h' is transposed with two full 128x128 PE transposes; h_stat [128,8] is a
    single stride-32 free-dim gather from the transpose PSUM; relu(h') lands in
    ht_full off the critical path.
  - out_w slice (bf16) is prefetched into SBUF during the recurrence, so the
    final logits matmul is compute-bound.
"""
import os
import sys
import numpy as np

for _p in ("/opt/trn_rl_repo", "/root/.axon_site/_ro/trn_rl_repo"):
    if os.path.isdir(_p) and _p not in sys.path:
        sys.path.insert(0, _p)

import concourse.bass as bass
import concourse.bacc as bacc
import concourse.tile as tile
import concourse.mybir as mybir
from concourse.bass_utils import run_bass_kernel_spmd
from concourse.masks import make_identity

F32 = mybir.dt.float32
BF16 = mybir.dt.bfloat16
AF = mybir.ActivationFunctionType
ALU = mybir.AluOpType
NP_BF16 = mybir.dt.np(BF16)

E = 1024          # emb dim
EC = 8            # E / 128 chunks
T = 65            # decode steps (1 SOS + 64)
L = 320           # context rows (128 + 64 + 128)
V0 = 32000
V = V0 + 2        # 32002
NCORES = 8
VP = 4096         # per-core padded vocab slice (8 * 4096 = 32768 >= 32002)
G = 4             # col-tile groups
RG = 768          # region width per group (3 gates x 256)

_CACHE = {}


def _arrange_w(w):
    """W_ih halves [3072, 1024] -> [128, 8*4*768]: out[p, ((c*4)+j)*768 + g*256+mm]
    = w[g*1024 + j*256 + mm, c*128 + p].  (c = standard 128-chunk of the input.)"""
    x = w.reshape(3, 4, 256, EC, 128)            # g, j, mm, c, p
    x = np.transpose(x, (4, 3, 1, 0, 2))         # p, c, j, g, mm
    return np.ascontiguousarray(x).reshape(128, EC * G * RG)


def _arrange_whh(w):
    """W_hh [3072, 1024] -> [128, 8*4*768] with the h-chunk dim in k-order
    (k = 4*half + jh, covering h[256*jh + 128*half + p]):
    out[p, (k*4 + j)*768 + g*256+mm] = w[g*1024 + j*256 + mm, 256*jh + 128*half + p]."""
    x = w.reshape(3, 4, 256, 4, 2, 128)          # g, j, mm, jh, half, p
    x = np.transpose(x, (5, 4, 3, 1, 0, 2))      # p, half, jh, j, g, mm
    return np.ascontiguousarray(x).reshape(128, EC * G * RG)


def _bias_tall(b_rzn):
    """[3072] bias in gate order -> [128, 768] with row 32j = region j."""
    x = b_rzn.reshape(3, 4, 256)                 # g, j, mm
    x = np.transpose(x, (1, 0, 2)).reshape(4, RG)  # j, (g mm)
    out = np.zeros((128, RG), np.float32)
    out[::32, :] = x
    return out


def build_program():
    nc = bacc.Bacc("TRN2", target_bir_lowering=False, debug=False, num_devices=NCORES)

    ctx_d = nc.dram_tensor("ctx", [L, E], F32, kind="ExternalInput").ap()
    decx_d = nc.dram_tensor("decx", [T, E], F32, kind="ExternalInput").ap()
    we_d = nc.dram_tensor("we", [1, E], F32, kind="ExternalInput").ap()
    whh_d = nc.dram_tensor("whh", [128, EC * G * RG], BF16, kind="ExternalInput").ap()
    wc_d = nc.dram_tensor("wc", [128, EC * G * RG], BF16, kind="ExternalInput").ap()
    wx_d = nc.dram_tensor("wx", [128, EC * G * RG], BF16, kind="ExternalInput").ap()
    bias_d = nc.dram_tensor("bias", [128, RG], F32, kind="ExternalInput").ap()
    h0k_d = nc.dram_tensor("h0k", [128, EC], BF16, kind="ExternalInput").ap()
    owt_d = nc.dram_tensor("owt", [128, EC * VP], BF16, kind="ExternalInput").ap()
    outb_d = nc.dram_tensor("outb", [1, VP], BF16, kind="ExternalInput").ap()
    out_d = nc.dram_tensor("out", [T, VP], F32, kind="ExternalOutput").ap()

    with tile.TileContext(nc) as tc:
        with tc.tile_pool(name="persist", bufs=1) as pp:
            # ---------- persistent constants ----------
            # (whh DMA is emitted after wc/wx so the small phase-1 loads and
            # the phase-2/3 weights win the DMA queues first)
            whh = pp.tile([128, EC * G * RG], BF16)

            ident = pp.tile([128, 128], F32)
            make_identity(nc, ident[:])
            ident_bf = pp.tile([128, 128], BF16)
            nc.vector.tensor_copy(ident_bf[:], ident[:])

            ones_tall = pp.tile([128, T], F32)
            nc.gpsimd.memset(ones_tall[:], 1.0)
            ones_col = pp.tile([128, 1], F32)
            nc.gpsimd.memset(ones_col[:], 1.0)
            ones_row = pp.tile([1, 128], F32)
            nc.gpsimd.memset(ones_row[:], 1.0)

            bias_tall = pp.tile([128, RG], F32)    # (b_ih + b_hh) rz | b_ih n, rows at 32j
            nc.sync.dma_start(bias_tall[:], bias_d[:])
            bias_bf = pp.tile([128, RG], BF16)
            nc.vector.tensor_copy(bias_bf[:], bias_tall[:])

            girz = pp.tile([T, G, 512], BF16)      # gi rz-part, partition = t
            gin_tall = pp.tile([128, T * 256], BF16)  # gi n-part at rows 32j
            gic_tall = pp.tile([128, RG], F32)     # const part of gi, rows at 32j
            ht_full = pp.tile([128, EC * T], BF16)  # [p, k, t] = relu(h_t[chunk k])
            h_stat = pp.tile([128, EC], BF16)      # h in k-order columns (MM stationary)
            cT_bf = pp.tile([128, EC], BF16)
            dxT_bf = pp.tile([128, EC, T], BF16)

            nc.gpsimd.memset(gin_tall[:], 0.0)
            # h0 in k-order: h_stat[p, half*4+jh] = decx[0, 256*jh+128*half+p]
            nc.sync.dma_start(h_stat[:], h0k_d[:])

            # ---------- phase 1: attention (constant across steps) ----------
            with tc.tile_pool(name="ph1", bufs=1) as p1, \
                 tc.tile_pool(name="ph1ps", bufs=1, space="PSUM") as p1ps:
                # dec_x first: phase 3 only needs dxT, so get it ready early
                decx_sb = p1.tile([T, E], F32)
                nc.sync.dma_start(decx_sb[:], decx_d[:])
                we_sb = p1.tile([1, E], F32)
                nc.sync.dma_start(we_sb[:], we_d[:])
                rows3 = (128, 128, 64)
                ctx_sb = []
                for i, rows in enumerate(rows3):
                    t_ = p1.tile([128, E], F32, tag=f"ctx{i}")
                    nc.sync.dma_start(t_[:rows, :], ctx_d[128 * i:128 * i + rows, :])
                    ctx_sb.append(t_)

                # dec_x^T [128, 8, 65] bf16 via PE transposes (bf16 in for speed)
                decx_bf = p1.tile([T, E], BF16)
                nc.vector.tensor_copy(decx_bf[:], decx_sb[:])
                dxT_ps = p1ps.tile([128, T], BF16, space="PSUM")
                for k in range(EC):
                    nc.tensor.transpose(out=dxT_ps[:, :],
                                        in_=decx_bf[:T, 128 * k:128 * (k + 1)],
                                        identity=ident_bf[:T, :T])
                    nc.vector.tensor_copy(dxT_bf[:, k, :], dxT_ps[:, :])

                # replicate w_e across partitions via K=1 matmul
                werep_ps = p1ps.tile([128, E], F32, space="PSUM")
                for half in range(2):
                    nc.tensor.matmul(werep_ps[:, 512 * half:512 * (half + 1)],
                                     lhsT=ones_row[:1, :],
                                     rhs=we_sb[:1, 512 * half:512 * (half + 1)],
                                     start=True, stop=True)
                werep = p1.tile([128, E], F32)
                nc.vector.tensor_copy(werep[:], werep_ps[:])

                scratch = p1.tile([128, E], F32)
                escore = [p1.tile([128, 1], F32, tag=f"esc{i}", name=f"esc{i}")
                          for i in range(3)]
                for i, rows in enumerate(rows3):
                    sc = p1.tile([128, 1], F32, tag=f"sc{i}")
                    nc.vector.tensor_tensor(out=scratch[:rows, :],
                                            in0=ctx_sb[i][:rows, :],
                                            in1=werep[:rows, :], op=ALU.mult)
                    nc.vector.tensor_reduce(out=sc[:rows, :], in_=scratch[:rows, :],
                                            axis=mybir.AxisListType.X, op=ALU.add)
                    nc.scalar.activation(escore[i][:rows, :], sc[:rows, :], AF.Exp)
                ssum_ps = p1ps.tile([1, 1], F32, space="PSUM")
                for i, rows in enumerate(rows3):
                    nc.tensor.matmul(ssum_ps[:1, :1], lhsT=escore[i][:rows, :1],
                                     rhs=ones_col[:rows, :1],
                                     start=(i == 0), stop=(i == 2))
                rsum = p1.tile([1, 1], F32)
                nc.vector.reciprocal(rsum[:], ssum_ps[:1, :1])

                cun_ps = p1ps.tile([1, E], F32, space="PSUM")
                for half in range(2):
                    for i, rows in enumerate(rows3):
                        nc.tensor.matmul(cun_ps[:1, 512 * half:512 * (half + 1)],
                                         lhsT=escore[i][:rows, :1],
                                         rhs=ctx_sb[i][:rows, 512 * half:512 * (half + 1)],
                                         start=(i == 0), stop=(i == 2))
                c_sb = p1.tile([1, E], F32)
                nc.vector.tensor_scalar_mul(c_sb[:], cun_ps[:1, :], rsum[:1, :1])

                # c^T [128, 8] bf16 via PE transposes
                cT_ps = p1ps.tile([128, EC], F32, space="PSUM")
                for k in range(EC):
                    nc.tensor.transpose(out=cT_ps[:, k:k + 1],
                                        in_=c_sb[:1, 128 * k:128 * (k + 1)],
                                        identity=ident[:1, :1])
                nc.vector.tensor_copy(cT_bf[:], cT_ps[:])

            # ---------- phase 2: gic = W_ih[:, :E] @ c + biases ----------
            with tc.tile_pool(name="pwc", bufs=1) as pwc, \
                 tc.tile_pool(name="pwcps", bufs=1, space="PSUM") as pwcps:
                wc_sb = pwc.tile([128, EC * G * RG], BF16)
                wcv = wc_sb[:].rearrange("p (c j m) -> p c j m", c=EC, j=G)
                for c in range(EC):
                    nc.sync.dma_start(wc_sb[:, c * G * RG:(c + 1) * G * RG],
                                      wc_d[:, c * G * RG:(c + 1) * G * RG])
                gic_ps = pwcps.tile([128, 1024], F32, space="PSUM")
                for c in range(EC):
                    for j in range(G):
                        nc.tensor.matmul(gic_ps[32 * j:32 * j + 1, 0:512],
                                         lhsT=cT_bf[:, c:c + 1], rhs=wcv[:, c, j, 0:512],
                                         start=(c == 0), stop=False,
                                         tile_position=(0, 32 * j))
                        nc.tensor.matmul(gic_ps[32 * j:32 * j + 1, 512:768],
                                         lhsT=cT_bf[:, c:c + 1], rhs=wcv[:, c, j, 512:768],
                                         start=(c == 0), stop=False,
                                         tile_position=(0, 32 * j))
                # + (b_ih + b_hh)_rz | b_ih_n  via K=1 matmul
                for j in range(G):
                    nc.tensor.matmul(gic_ps[32 * j:32 * j + 1, 0:512],
                                     lhsT=ident_bf[32 * j:32 * j + 1, 32 * j:32 * j + 1],
                                     rhs=bias_bf[32 * j:32 * j + 1, 0:512],
                                     start=False, stop=True,
                                     tile_position=(32 * j, 32 * j))
                    nc.tensor.matmul(gic_ps[32 * j:32 * j + 1, 512:768],
                                     lhsT=ident_bf[32 * j:32 * j + 1, 32 * j:32 * j + 1],
                                     rhs=bias_bf[32 * j:32 * j + 1, 512:768],
                                     start=False, stop=True,
                                     tile_position=(32 * j, 32 * j))
                for j in range(G):
                    if j % 2 == 0:
                        nc.scalar.copy(gic_tall[32 * j:32 * j + 1, :],
                                       gic_ps[32 * j:32 * j + 1, 0:RG])
                    else:
                        nc.vector.tensor_copy(gic_tall[32 * j:32 * j + 1, :],
                                              gic_ps[32 * j:32 * j + 1, 0:RG])

            # ---------- phase 3: gi[t] = gic + W_ih[:, E:] @ x_t ----------
            with tc.tile_pool(name="pwx", bufs=1) as pwx, \
                 tc.tile_pool(name="pwxps", bufs=1, space="PSUM") as pwxps:
                wx_sb = pwx.tile([128, EC * G * RG], BF16)
                wxv = wx_sb[:].rearrange("p (c j m) -> p c j m", c=EC, j=G)
                # chunked DMA so the c-chunk matmuls overlap the load
                for c in range(EC):
                    nc.sync.dma_start(wx_sb[:, c * G * RG:(c + 1) * G * RG],
                                      wx_d[:, c * G * RG:(c + 1) * G * RG])
                nc.sync.dma_start(whh[:], whh_d[:])
                rz_ps = [pwxps.tile([T, 512], F32, space="PSUM", tag=f"girz{j}",
                                    name=f"girz{j}") for j in range(G)]
                n_ps = [pwxps.tile([T, 256], F32, space="PSUM", tag=f"gin{j}",
                                   name=f"gin{j}") for j in range(G)]
                for c in range(EC):
                    for j in range(G):
                        nc.tensor.matmul(rz_ps[j][:T, :], lhsT=dxT_bf[:, c, :],
                                         rhs=wxv[:, c, j, 0:512],
                                         start=(c == 0), stop=False)
                        nc.tensor.matmul(n_ps[j][:T, :], lhsT=dxT_bf[:, c, :],
                                         rhs=wxv[:, c, j, 512:768],
                                         start=(c == 0), stop=False)
                for j in range(G):
                    nc.tensor.matmul(rz_ps[j][:T, :],
                                     lhsT=ones_tall[32 * j:32 * j + 1, :T],
                                     rhs=gic_tall[32 * j:32 * j + 1, 0:512],
                                     start=False, stop=True,
                                     tile_position=(32 * j, 0))
                    nc.tensor.matmul(n_ps[j][:T, :],
                                     lhsT=ones_tall[32 * j:32 * j + 1, :T],
                                     rhs=gic_tall[32 * j:32 * j + 1, 512:768],
                                     start=False, stop=True,
                                     tile_position=(32 * j, 0))
                    nc.vector.tensor_copy(girz[:, j, :], rz_ps[j][:T, :])
                    nbf = pwx.tile([T, 256], BF16, tag="ginbf")
                    nc.vector.tensor_copy(nbf[:], n_ps[j][:T, :])
                    nc.sync.dma_start(gin_tall[32 * j:32 * j + 1, :], nbf[:T, :])

            # ---------- phase 4: the 65-step recurrence ----------
            whhv = whh[:].rearrange("p (k j m) -> p k j m", k=EC, j=G)
            ginv = gin_tall[:].rearrange("p (t m) -> p t m", t=T)
            htv = ht_full[:].rearrange("p (k t) -> p k t", k=EC)
            with tc.tile_pool(name="recsb", bufs=1) as prl, \
                 tc.tile_pool(name="rec", bufs=2) as pr, \
                 tc.tile_pool(name="recps_g", bufs=1, space="PSUM") as prg, \
                 tc.tile_pool(name="recps_t", bufs=1, space="PSUM") as prs:
                # prefetch out_w^T slice during the recurrence (queued after wx)
                owt_sb = prl.tile([128, EC * VP], BF16)
                nc.sync.dma_start(owt_sb[:], owt_d[:])
                outb_sb = prl.tile([1, VP], BF16)
                nc.sync.dma_start(outb_sb[:], outb_d[:])
                ones_bf = prl.tile([1, T], BF16)
                nc.vector.tensor_copy(ones_bf[:], ones_tall[0:1, :])
                psgs = [prg.tile([128, 768], F32, space="PSUM", tag=f"psg{t % 2}",
                                 name=f"psg{t % 2}")
                        for t in range(2)]
                # trash bank for heartbeat matmuls that keep the PE HAM at
                # K=8/8 through the serial gate tail (results never read);
                # reading rz_sb pins each batch into its own step's window
                trash = prg.tile([1, 512], F32, space="PSUM", tag="trash")

                def heartbeat(n_mm, src):
                    for _ in range(n_mm):
                        nc.tensor.matmul(trash[0:1, 0:512],
                                         lhsT=ident_bf[:, 0:1],
                                         rhs=src,
                                         start=True, stop=True,
                                         skip_group_check=True)
                # pre-fold gi_rz[0] into psum 0
                for j in range(G):
                    nc.tensor.matmul(psgs[0][32 * j:32 * j + 1, 0:512],
                                     lhsT=ident_bf[:T, 0:1], rhs=girz[:T, j, :],
                                     start=True, stop=False, tile_position=(0, 32 * j))
                for t in range(T):
                    psg = psgs[t % 2]
                    # rz matvec: c-outer / j-inner so the 4 col strips overlap
                    for c in range(EC):
                        for j in range(G):
                            nc.tensor.matmul(psg[32 * j:32 * j + 1, 0:512],
                                             lhsT=h_stat[:, c:c + 1],
                                             rhs=whhv[:, c, j, 0:512],
                                             start=False, stop=(c == EC - 1),
                                             tile_position=(0, 32 * j))
                    # n matvec
                    for c in range(EC):
                        for j in range(G):
                            nc.tensor.matmul(psg[32 * j:32 * j + 1, 512:768],
                                             lhsT=h_stat[:, c:c + 1],
                                             rhs=whhv[:, c, j, 512:768],
                                             start=(c == 0), stop=(c == EC - 1),
                                             tile_position=(0, 32 * j))
                    # pre-fold gi_rz[t+1] into the alternate psum (runs in tail idle)
                    if t + 1 < T:
                        nxt = psgs[(t + 1) % 2]
                        for j in range(G):
                            nc.tensor.matmul(nxt[32 * j:32 * j + 1, 0:512],
                                             lhsT=ident_bf[:T, t + 1:t + 2],
                                             rhs=girz[:T, j, :],
                                             start=True, stop=False,
                                             tile_position=(0, 32 * j))

                    # gate tail on full-width tiles (junk rows are harmless).
                    # h' = z*h + (1-z)*n, computed entirely in transposed
                    # [128, 8] "k-space" so no wide h state is kept at all.
                    rz_sb = pr.tile([128, 512], BF16, tag="rz")
                    nc.scalar.activation(rz_sb[:], psg[:, 0:512], AF.Sigmoid)
                    t1 = pr.tile([128, 256], BF16, tag="t1")
                    nc.vector.tensor_tensor(out=t1[:], in0=rz_sb[:, 0:256],
                                            in1=psg[:, 512:768], op=ALU.mult)
                    npre = pr.tile([128, 256], BF16, tag="npre")
                    nc.vector.tensor_tensor(out=npre[:], in0=t1[:],
                                            in1=ginv[:, t, :], op=ALU.add)
                    # transpose z and npre to k-space (PE idles here anyway);
                    # both live in one 2KB psum bank
                    tpzn = prs.tile([128, 1024], BF16, space="PSUM", tag="tpzn")
                    nc.tensor.transpose(out=tpzn[:, 0:128], in_=rz_sb[:, 256:384],
                                        identity=ident_bf[:, :])
                    nc.tensor.transpose(out=tpzn[:, 128:256], in_=rz_sb[:, 384:512],
                                        identity=ident_bf[:, :])
                    nc.tensor.transpose(out=tpzn[:, 256:384], in_=npre[:, 0:128],
                                        identity=ident_bf[:, :])
                    nc.tensor.transpose(out=tpzn[:, 384:512], in_=npre[:, 128:256],
                                        identity=ident_bf[:, :])
                    heartbeat(3, rz_sb[:, 0:512])
                    tpzv = tpzn[:, 0:256].rearrange("p (k s) -> p k s", k=EC)
                    tpnv = tpzn[:, 256:512].rearrange("p (k s) -> p k s", k=EC)
                    # off-chain: zh = z*h, oz = 1-z   (both [128, 8] k-space)
                    zh_k = pr.tile([128, EC], BF16, tag="zh_k")
                    nc.vector.tensor_tensor(out=zh_k[:], in0=h_stat[:],
                                            in1=tpzv[:, :, 0], op=ALU.mult)
                    oz_k = pr.tile([128, EC], BF16, tag="oz_k")
                    nc.scalar.activation(oz_k[:], tpzv[:, :, 0], AF.Identity,
                                         bias=1.0, scale=-1.0)
                    # chain: tanh -> m1 = (1-z)*n -> h' = m1 + zh
                    n_k = pr.tile([128, EC], BF16, tag="n_k")
                    nc.scalar.activation(n_k[:], tpnv[:, :, 0], AF.Tanh)
                    m1 = pr.tile([128, EC], BF16, tag="m1")
                    nc.vector.tensor_tensor(out=m1[:], in0=oz_k[:], in1=n_k[:],
                                            op=ALU.mult)
                    nc.vector.tensor_tensor(out=h_stat[:], in0=m1[:], in1=zh_k[:],
                                            op=ALU.add)
                    nc.scalar.activation(htv[:, :, t:t + 1],
                                         h_stat[:].unsqueeze(2), AF.Relu)

                # ------ phase 5: logits = relu(H) @ out_w^T + out_b ------
                owtv = owt_sb[:].rearrange("p (k v) -> p k v", k=EC)
                with tc.tile_pool(name="fin", bufs=2) as pf, \
                     tc.tile_pool(name="finps", bufs=2, space="PSUM") as pfps:
                    for vb in range(VP // 512):
                        ops = pfps.tile([T, 512], F32, space="PSUM", tag="ops")
                        for k in range(EC):
                            nc.tensor.matmul(ops[:T, :], lhsT=htv[:, k, :],
                                             rhs=owtv[:, k, 512 * vb:512 * (vb + 1)],
                                             start=(k == 0), stop=False)
                        nc.tensor.matmul(ops[:T, :], lhsT=ones_bf[:1, :T],
                                         rhs=outb_sb[:1, 512 * vb:512 * (vb + 1)],
                                         start=False, stop=True)
                        osb = pf.tile([T, 512], F32, tag="osb")
                        nc.vector.tensor_copy(osb[:], ops[:T, :])
                        nc.sync.dma_start(out_d[:, 512 * vb:512 * (vb + 1)], osb[:])

    nc.compile()
    return nc


def _prep_inputs(inp):
    idx_enc = np.concatenate([inp["input_diagnosis"], inp["input_procedure"],
                              inp["input_medicine"]]).astype(np.int64)
    tokens = np.concatenate([np.array([V0], np.int64),
                             inp["dec_tokens"].astype(np.int64)])
    enc_emb = np.asarray(inp["enc_emb"], np.float32)
    dec_emb = np.asarray(inp["dec_emb"], np.float32)

    ctx = np.ascontiguousarray(enc_emb[idx_enc])                       # [320, 1024]
    decx = np.ascontiguousarray(dec_emb[tokens])                       # [65, 1024]
    we = np.ascontiguousarray(np.asarray(inp["attn_w"], np.float32)[0, E:]).reshape(1, E)

    w_ih = np.asarray(inp["gru_w_ih"], np.float32)                     # [3072, 2048]
    w_hh = np.asarray(inp["gru_w_hh"], np.float32)                     # [3072, 1024]
    b_ih = np.asarray(inp["gru_b_ih"], np.float32)
    b_hh = np.asarray(inp["gru_b_hh"], np.float32)
    assert not np.any(b_hh[2 * E:]), "nonzero b_hh n-gate not supported on device"

    whh_arr = _arrange_whh(w_hh).astype(NP_BF16)                       # [128, 24576]
    wc_arr = _arrange_w(np.ascontiguousarray(w_ih[:, :E])).astype(NP_BF16)
    wx_arr = _arrange_w(np.ascontiguousarray(w_ih[:, E:])).astype(NP_BF16)
    bias = b_ih.copy()
    bias[:2 * E] += b_hh[:2 * E]
    bias_arr = _bias_tall(bias)                                        # [128, 768] f32

    out_w = np.asarray(inp["out_w"], np.float32)
    out_b = np.asarray(inp["out_b"], np.float32)
    owp = np.zeros((NCORES * VP, E), np.float32)
    owp[:V] = out_w
    obp = np.zeros(NCORES * VP, np.float32)
    obp[:V] = out_b

    # h0 in k-order columns: h0k[p, 4*half + jh] = decx[0, 256*jh + 128*half + p]
    h0k = np.ascontiguousarray(
        np.transpose(decx[0].reshape(4, 2, 128), (2, 1, 0))).reshape(128, EC).astype(NP_BF16)

    base = {"ctx": ctx, "decx": decx, "we": we, "whh": whh_arr,
            "wc": wc_arr, "wx": wx_arr, "bias": bias_arr, "h0k": h0k}
    in_maps = []
    for i in range(NCORES):
        s = owp[i * VP:(i + 1) * VP]                                   # [4096, 1024]
        # owt[p, k*VP + v] = s[v, 256*(k%4) + 128*(k//4) + p]
        x = s.reshape(VP, 4, 2, 128)                                   # v, jh, half, p
        owt = np.ascontiguousarray(
            np.transpose(x, (3, 2, 1, 0))).astype(NP_BF16).reshape(128, EC * VP)
        m = dict(base)
        m["owt"] = owt
        m["outb"] = np.ascontiguousarray(
            obp[i * VP:(i + 1) * VP]).reshape(1, VP).astype(NP_BF16)
        in_maps.append(m)
    return in_maps


def kernel(**inputs):
    if "nc" not in _CACHE:
        _CACHE["nc"] = build_program()
    nc = _CACHE["nc"]
    in_maps = _prep_inputs({k: np.asarray(v) for k, v in inputs.items()})
    res = run_bass_kernel_spmd(nc, in_maps, core_ids=list(range(NCORES)))
    slices = [res.results[i]["out"] for i in range(NCORES)]            # each [65, 4096]
    logits = np.concatenate(slices, axis=1)[:, :V]
    return np.ascontiguousarray(logits.astype(np.float32))


# revision 39
# speedup vs baseline: 6.9023x; 1.1156x over previous
"""Trainium2 Bass kernel for nn_LEAP_74371653697613 (GRU decoder w/ additive attention).

Structure exploited:
  - softmax(ctx_score + h.w_h + b) == softmax(ctx_score): attention weights are
    constant across decode steps -> context vector c computed once on device.
  - gi_t = W_ih @ [c; x_t] + b_ih is teacher-forced -> batched matmuls, precomputed.
  - logits don't feed back -> one big relu(H) @ out_w^T matmul at the end,
    vocab-sharded across the 8 cores (each core gets a 4096-row slice of out_w).
  - only W_hh @ h_t + gates is sequential (65 steps); it runs identically on all
    8 cores (replicated -> zero cross-core communication).

Per-step device schedule (v2):
  - 4 col-tiled groups (tile_position=(0,32j)) of bf16 matvec MMs, emitted
    c-outer/j-inner so the four 32-col strips of the PE array run concurrently;
    gi_rz[t] folded into the same PSUM accumulation via a one-hot K=65 matmul
    issued one step ahead (hides in the tail of step t-1).
  - gate math runs on FULL 128-partition tiles straight out of PSUM (rows other
    than {0,32,64,96} compute harmless garbage; engines cost by elems-per-lane,
    so the wide op is the same price and needs no bridge/compaction).
  - h' is transposed with two full 128x128 PE transposes; h_stat [128,8] is a
    single stride-32 free-dim gather from the transpose PSUM; relu(h') lands in
    ht_full off the critical path.
  - out_w slice (bf16) is prefetched into SBUF during the recurrence, so the
    final logits matmul is compute-bound.
"""
import os
import sys
import numpy as np

for _p in ("/opt/trn_rl_repo", "/root/.axon_site/_ro/trn_rl_repo"):
    if os.path.isdir(_p) and _p not in sys.path:
        sys.path.insert(0, _p)

import concourse.bass as bass
import concourse.bacc as bacc
import concourse.tile as tile
import concourse.mybir as mybir
from concourse.bass_utils import run_bass_kernel_spmd
from concourse.masks import make_identity

F32 = mybir.dt.float32
BF16 = mybir.dt.bfloat16
AF = mybir.ActivationFunctionType
ALU = mybir.AluOpType
NP_BF16 = mybir.dt.np(BF16)

E = 1024          # emb dim
EC = 8            # E / 128 chunks
T = 65            # decode steps (1 SOS + 64)
L = 320           # context rows (128 + 64 + 128)
V0 = 32000
V = V0 + 2        # 32002
NCORES = 8
VP = 4096         # per-core padded vocab slice (8 * 4096 = 32768 >= 32002)
G = 4             # col-tile groups
RG = 768          # region width per group (3 gates x 256)

_CACHE = {}


def _arrange_w(w):
    """W_ih halves [3072, 1024] -> [128, 8*4*768]: out[p, ((c*4)+j)*768 + g*256+mm]
    = w[g*1024 + j*256 + mm, c*128 + p].  (c = standard 128-chunk of the input.)"""
    x = w.reshape(3, 4, 256, EC, 128)            # g, j, mm, c, p
    x = np.transpose(x, (4, 3, 1, 0, 2))         # p, c, j, g, mm
    return np.ascontiguousarray(x).reshape(128, EC * G * RG)


def _arrange_whh(w):
    """W_hh [3072, 1024] -> [128, 8*4*768] with the h-chunk dim in k-order
    (k = 4*half + jh, covering h[256*jh + 128*half + p]):
    out[p, (k*4 + j)*768 + g*256+mm] = w[g*1024 + j*256 + mm, 256*jh + 128*half + p]."""
    x = w.reshape(3, 4, 256, 4, 2, 128)          # g, j, mm, jh, half, p
    x = np.transpose(x, (5, 4, 3, 1, 0, 2))      # p, half, jh, j, g, mm
    return np.ascontiguousarray(x).reshape(128, EC * G * RG)


def _bias_tall(b_rzn):
    """[3072] bias in gate order -> [128, 768] with row 32j = region j."""
    x = b_rzn.reshape(3, 4, 256)                 # g, j, mm
    x = np.transpose(x, (1, 0, 2)).reshape(4, RG)  # j, (g mm)
    out = np.zeros((128, RG), np.float32)
    out[::32, :] = x
    return out


def build_program():
    nc = bacc.Bacc("TRN2", target_bir_lowering=False, debug=False, num_devices=NCORES)

    ctx_d = nc.dram_tensor("ctx", [L, E], F32, kind="ExternalInput").ap()
    decx_d = nc.dram_tensor("decx", [T, E], F32, kind="ExternalInput").ap()
    we_d = nc.dram_tensor("we", [1, E], F32, kind="ExternalInput").ap()
    whh_d = nc.dram_tensor("whh", [128, EC * G * RG], BF16, kind="ExternalInput").ap()
    wc_d = nc.dram_tensor("wc", [128, EC * G * RG], BF16, kind="ExternalInput").ap()
    wx_d = nc.dram_tensor("wx", [128, EC * G * RG], BF16, kind="ExternalInput").ap()
    bias_d = nc.dram_tensor("bias", [128, RG], F32, kind="ExternalInput").ap()
    h0k_d = nc.dram_tensor("h0k", [128, EC], BF16, kind="ExternalInput").ap()
    owt_d = nc.dram_tensor("owt", [128, EC * VP], BF16, kind="ExternalInput").ap()
    outb_d = nc.dram_tensor("outb", [1, VP], BF16, kind="ExternalInput").ap()
    out_d = nc.dram_tensor("out", [T, VP], F32, kind="ExternalOutput").ap()

    with tile.TileContext(nc) as tc:
        with tc.tile_pool(name="persist", bufs=1) as pp:
            # ---------- persistent constants ----------
            # (whh DMA is emitted after wc/wx so the small phase-1 loads and
            # the phase-2/3 weights win the DMA queues first)
            whh = pp.tile([128, EC * G * RG], BF16)

            ident = pp.tile([128, 128], F32)
            make_identity(nc, ident[:])
            ident_bf = pp.tile([128, 128], BF16)
            nc.vector.tensor_copy(ident_bf[:], ident[:])

            ones_tall = pp.tile([128, T], F32)
            nc.gpsimd.memset(ones_tall[:], 1.0)
            ones_col = pp.tile([128, 1], F32)
            nc.gpsimd.memset(ones_col[:], 1.0)
            ones_row = pp.tile([1, 128], F32)
            nc.gpsimd.memset(ones_row[:], 1.0)

            bias_tall = pp.tile([128, RG], F32)    # (b_ih + b_hh) rz | b_ih n, rows at 32j
            nc.sync.dma_start(bias_tall[:], bias_d[:])
            bias_bf = pp.tile([128, RG], BF16)
            nc.vector.tensor_copy(bias_bf[:], bias_tall[:])

            girz = pp.tile([T, G, 512], BF16)      # gi rz-part, partition = t
            gin_tall = pp.tile([128, T * 256], BF16)  # gi n-part at rows 32j
            gic_tall = pp.tile([128, RG], F32)     # const part of gi, rows at 32j
            ht_full = pp.tile([128, EC * T], BF16)  # [p, k, t] = relu(h_t[chunk k])
            h_stat = pp.tile([128, EC], BF16)      # h in k-order columns (MM stationary)
            cT_bf = pp.tile([128, EC], BF16)
            dxT_bf = pp.tile([128, EC, T], BF16)

            nc.gpsimd.memset(gin_tall[:], 0.0)
            # h0 in k-order: h_stat[p, half*4+jh] = decx[0, 256*jh+128*half+p]
            nc.sync.dma_start(h_stat[:], h0k_d[:])

            # ---------- phase 1: attention (constant across steps) ----------
            with tc.tile_pool(name="ph1", bufs=1) as p1, \
                 tc.tile_pool(name="ph1ps", bufs=1, space="PSUM") as p1ps:
                # dec_x first: phase 3 only needs dxT, so get it ready early
                decx_sb = p1.tile([T, E], F32)
                nc.sync.dma_start(decx_sb[:], decx_d[:])
                we_sb = p1.tile([1, E], F32)
                nc.sync.dma_start(we_sb[:], we_d[:])
                rows3 = (128, 128, 64)
                ctx_sb = []
                for i, rows in enumerate(rows3):
                    t_ = p1.tile([128, E], F32, tag=f"ctx{i}")
                    nc.sync.dma_start(t_[:rows, :], ctx_d[128 * i:128 * i + rows, :])
                    ctx_sb.append(t_)

                # dec_x^T [128, 8, 65] bf16 via PE transposes (bf16 in for speed)
                decx_bf = p1.tile([T, E], BF16)
                nc.vector.tensor_copy(decx_bf[:], decx_sb[:])
                dxT_ps = p1ps.tile([128, T], BF16, space="PSUM")
                for k in range(EC):
                    nc.tensor.transpose(out=dxT_ps[:, :],
                                        in_=decx_bf[:T, 128 * k:128 * (k + 1)],
                                        identity=ident_bf[:T, :T])
                    nc.vector.tensor_copy(dxT_bf[:, k, :], dxT_ps[:, :])

                # replicate w_e across partitions via K=1 matmul
                werep_ps = p1ps.tile([128, E], F32, space="PSUM")
                for half in range(2):
                    nc.tensor.matmul(werep_ps[:, 512 * half:512 * (half + 1)],
                                     lhsT=ones_row[:1, :],
                                     rhs=we_sb[:1, 512 * half:512 * (half + 1)],
                                     start=True, stop=True)
                werep = p1.tile([128, E], F32)
                nc.vector.tensor_copy(werep[:], werep_ps[:])

                scratch = p1.tile([128, E], F32)
                escore = [p1.tile([128, 1], F32, tag=f"esc{i}", name=f"esc{i}")
                          for i in range(3)]
                for i, rows in enumerate(rows3):
                    sc = p1.tile([128, 1], F32, tag=f"sc{i}")
                    nc.vector.tensor_tensor(out=scratch[:rows, :],
                                            in0=ctx_sb[i][:rows, :],
                                            in1=werep[:rows, :], op=ALU.mult)
                    nc.vector.tensor_reduce(out=sc[:rows, :], in_=scratch[:rows, :],
                                            axis=mybir.AxisListType.X, op=ALU.add)
                    nc.scalar.activation(escore[i][:rows, :], sc[:rows, :], AF.Exp)
                ssum_ps = p1ps.tile([1, 1], F32, space="PSUM")
                for i, rows in enumerate(rows3):
                    nc.tensor.matmul(ssum_ps[:1, :1], lhsT=escore[i][:rows, :1],
                                     rhs=ones_col[:rows, :1],
                                     start=(i == 0), stop=(i == 2))
                rsum = p1.tile([1, 1], F32)
                nc.vector.reciprocal(rsum[:], ssum_ps[:1, :1])

                cun_ps = p1ps.tile([1, E], F32, space="PSUM")
                for half in range(2):
                    for i, rows in enumerate(rows3):
                        nc.tensor.matmul(cun_ps[:1, 512 * half:512 * (half + 1)],
                                         lhsT=escore[i][:rows, :1],
                                         rhs=ctx_sb[i][:rows, 512 * half:512 * (half + 1)],
                                         start=(i == 0), stop=(i == 2))
                c_sb = p1.tile([1, E], F32)
                nc.vector.tensor_scalar_mul(c_sb[:], cun_ps[:1, :], rsum[:1, :1])

                # c^T [128, 8] bf16 via PE transposes
                cT_ps = p1ps.tile([128, EC], F32, space="PSUM")
                for k in range(EC):
                    nc.tensor.transpose(out=cT_ps[:, k:k + 1],
                                        in_=c_sb[:1, 128 * k:128 * (k + 1)],
                                        identity=ident[:1, :1])
                nc.vector.tensor_copy(cT_bf[:], cT_ps[:])

            # ---------- phase 2: gic = W_ih[:, :E] @ c + biases ----------
            with tc.tile_pool(name="pwc", bufs=1) as pwc, \
                 tc.tile_pool(name="pwcps", bufs=1, space="PSUM") as pwcps:
                wc_sb = pwc.tile([128, EC * G * RG], BF16)
                wcv = wc_sb[:].rearrange("p (c j m) -> p c j m", c=EC, j=G)
                for c in range(EC):
                    nc.sync.dma_start(wc_sb[:, c * G * RG:(c + 1) * G * RG],
                                      wc_d[:, c * G * RG:(c + 1) * G * RG])
                gic_ps = pwcps.tile([128, 1024], F32, space="PSUM")
                for c in range(EC):
                    for j in range(G):
                        nc.tensor.matmul(gic_ps[32 * j:32 * j + 1, 0:512],
                                         lhsT=cT_bf[:, c:c + 1], rhs=wcv[:, c, j, 0:512],
                                         start=(c == 0), stop=False,
                                         tile_position=(0, 32 * j))
                        nc.tensor.matmul(gic_ps[32 * j:32 * j + 1, 512:768],
                                         lhsT=cT_bf[:, c:c + 1], rhs=wcv[:, c, j, 512:768],
                                         start=(c == 0), stop=False,
                                         tile_position=(0, 32 * j))
                # + (b_ih + b_hh)_rz | b_ih_n  via K=1 matmul
                for j in range(G):
                    nc.tensor.matmul(gic_ps[32 * j:32 * j + 1, 0:512],
                                     lhsT=ident_bf[32 * j:32 * j + 1, 32 * j:32 * j + 1],
                                     rhs=bias_bf[32 * j:32 * j + 1, 0:512],
                                     start=False, stop=True,
                                     tile_position=(32 * j, 32 * j))
                    nc.tensor.matmul(gic_ps[32 * j:32 * j + 1, 512:768],
                                     lhsT=ident_bf[32 * j:32 * j + 1, 32 * j:32 * j + 1],
                                     rhs=bias_bf[32 * j:32 * j + 1, 512:768],
                                     start=False, stop=True,
                                     tile_position=(32 * j, 32 * j))
                for j in range(G):
                    if j % 2 == 0:
                        nc.scalar.copy(gic_tall[32 * j:32 * j + 1, :],
                                       gic_ps[32 * j:32 * j + 1, 0:RG])
                    else:
                        nc.vector.tensor_copy(gic_tall[32 * j:32 * j + 1, :],
                                              gic_ps[32 * j:32 * j + 1, 0:RG])

            # ---------- phase 3: gi[t] = gic + W_ih[:, E:] @ x_t ----------
            with tc.tile_pool(name="pwx", bufs=1) as pwx, \
                 tc.tile_pool(name="pwxps", bufs=1, space="PSUM") as pwxps:
                wx_sb = pwx.tile([128, EC * G * RG], BF16)
                wxv = wx_sb[:].rearrange("p (c j m) -> p c j m", c=EC, j=G)
                # chunked DMA so the c-chunk matmuls overlap the load
                for c in range(EC):
                    nc.sync.dma_start(wx_sb[:, c * G * RG:(c + 1) * G * RG],
                                      wx_d[:, c * G * RG:(c + 1) * G * RG])
                nc.sync.dma_start(whh[:], whh_d[:])
                rz_ps = [pwxps.tile([T, 512], F32, space="PSUM", tag=f"girz{j}",
                                    name=f"girz{j}") for j in range(G)]
                n_ps = [pwxps.tile([T, 256], F32, space="PSUM", tag=f"gin{j}",
                                   name=f"gin{j}") for j in range(G)]
                for c in range(EC):
                    for j in range(G):
                        nc.tensor.matmul(rz_ps[j][:T, :], lhsT=dxT_bf[:, c, :],
                                         rhs=wxv[:, c, j, 0:512],
                                         start=(c == 0), stop=False)
                        nc.tensor.matmul(n_ps[j][:T, :], lhsT=dxT_bf[:, c, :],
                                         rhs=wxv[:, c, j, 512:768],
                                         start=(c == 0), stop=False)
                for j in range(G):
                    nc.tensor.matmul(rz_ps[j][:T, :],
                                     lhsT=ones_tall[32 * j:32 * j + 1, :T],
                                     rhs=gic_tall[32 * j:32 * j + 1, 0:512],
                                     start=False, stop=True,
                                     tile_position=(32 * j, 0))
                    nc.tensor.matmul(n_ps[j][:T, :],
                                     lhsT=ones_tall[32 * j:32 * j + 1, :T],
                                     rhs=gic_tall[32 * j:32 * j + 1, 512:768],
                                     start=False, stop=True,
                                     tile_position=(32 * j, 0))
                    nc.vector.tensor_copy(girz[:, j, :], rz_ps[j][:T, :])
                    nbf = pwx.tile([T, 256], BF16, tag="ginbf")
                    nc.vector.tensor_copy(nbf[:], n_ps[j][:T, :])
                    nc.sync.dma_start(gin_tall[32 * j:32 * j + 1, :], nbf[:T, :])

            # ---------- phase 4: the 65-step recurrence ----------
            whhv = whh[:].rearrange("p (k j m) -> p k j m", k=EC, j=G)
            ginv = gin_tall[:].rearrange("p (t m) -> p t m", t=T)
            htv = ht_full[:].rearrange("p (k t) -> p k t", k=EC)
            with tc.tile_pool(name="recsb", bufs=1) as prl, \
                 tc.tile_pool(name="rec", bufs=2) as pr, \
                 tc.tile_pool(name="recps_g", bufs=1, space="PSUM") as prg, \
                 tc.tile_pool(name="recps_t", bufs=1, space="PSUM") as prs:
                # prefetch out_w^T slice during the recurrence (queued after wx)
                owt_sb = prl.tile([128, EC * VP], BF16)
                nc.sync.dma_start(owt_sb[:], owt_d[:])
                outb_sb = prl.tile([1, VP], BF16)
                nc.sync.dma_start(outb_sb[:], outb_d[:])
                ones_bf = prl.tile([1, T], BF16)
                nc.vector.tensor_copy(ones_bf[:], ones_tall[0:1, :])
                psgs = [prg.tile([128, 768], F32, space="PSUM", tag=f"psg{t % 2}",
                                 name=f"psg{t % 2}")
                        for t in range(2)]
                # heartbeat matmuls keep the PE HAM at K=8/8 through the
                # serial gate tail; they scribble over the already-consumed
                # rz region of the current psg (the next user re-clears it
                # with start=True), so no extra psum bank is needed.
                def heartbeat(n_mm, dst, src):
                    for _ in range(n_mm):
                        nc.tensor.matmul(dst[0:1, 0:512],
                                         lhsT=ident_bf[:, 0:1],
                                         rhs=src,
                                         start=True, stop=True,
                                         skip_group_check=True)
                # pre-fold gi_rz[0] into psum 0
                for j in range(G):
                    nc.tensor.matmul(psgs[0][32 * j:32 * j + 1, 0:512],
                                     lhsT=ident_bf[:T, 0:1], rhs=girz[:T, j, :],
                                     start=True, stop=False, tile_position=(0, 32 * j))
                for t in range(T):
                    psg = psgs[t % 2]
                    # rz matvec: c-outer / j-inner so the 4 col strips overlap
                    for c in range(EC):
                        for j in range(G):
                            nc.tensor.matmul(psg[32 * j:32 * j + 1, 0:512],
                                             lhsT=h_stat[:, c:c + 1],
                                             rhs=whhv[:, c, j, 0:512],
                                             start=False, stop=(c == EC - 1),
                                             tile_position=(0, 32 * j))
                    # n matvec
                    for c in range(EC):
                        for j in range(G):
                            nc.tensor.matmul(psg[32 * j:32 * j + 1, 512:768],
                                             lhsT=h_stat[:, c:c + 1],
                                             rhs=whhv[:, c, j, 512:768],
                                             start=(c == 0), stop=(c == EC - 1),
                                             tile_position=(0, 32 * j))
                    # pre-fold gi_rz[t+1] into the alternate psum (runs in tail idle)
                    if t + 1 < T:
                        nxt = psgs[(t + 1) % 2]
                        for j in range(G):
                            nc.tensor.matmul(nxt[32 * j:32 * j + 1, 0:512],
                                             lhsT=ident_bf[:T, t + 1:t + 2],
                                             rhs=girz[:T, j, :],
                                             start=True, stop=False,
                                             tile_position=(0, 32 * j))

                    # gate tail on full-width tiles (junk rows are harmless).
                    # h' = z*h + (1-z)*n, computed entirely in transposed
                    # [128, 8] "k-space" so no wide h state is kept at all.
                    rz_sb = pr.tile([128, 512], BF16, tag="rz")
                    nc.scalar.activation(rz_sb[:], psg[:, 0:512], AF.Sigmoid)
                    t1 = pr.tile([128, 256], BF16, tag="t1")
                    nc.vector.tensor_tensor(out=t1[:], in0=rz_sb[:, 0:256],
                                            in1=psg[:, 512:768], op=ALU.mult)
                    npre = pr.tile([128, 256], BF16, tag="npre")
                    nc.vector.tensor_tensor(out=npre[:], in0=t1[:],
                                            in1=ginv[:, t, :], op=ALU.add)
                    # transpose z and npre to k-space (PE idles here anyway);
                    # separate tiles so tpz readers don't wait on tpn writes
                    tpz = prs.tile([128, 256], BF16, space="PSUM", tag="tpz")
                    nc.tensor.transpose(out=tpz[:, 0:128], in_=rz_sb[:, 256:384],
                                        identity=ident_bf[:, :])
                    nc.tensor.transpose(out=tpz[:, 128:256], in_=rz_sb[:, 384:512],
                                        identity=ident_bf[:, :])
                    tpn = prs.tile([128, 256], BF16, space="PSUM", tag="tpn")
                    nc.tensor.transpose(out=tpn[:, 0:128], in_=npre[:, 0:128],
                                        identity=ident_bf[:, :])
                    nc.tensor.transpose(out=tpn[:, 128:256], in_=npre[:, 128:256],
                                        identity=ident_bf[:, :])
                    heartbeat(3, psg, whh[:, 0:512])
                    tpzv = tpz[:].rearrange("p (k s) -> p k s", k=EC)
                    tpnv = tpn[:].rearrange("p (k s) -> p k s", k=EC)
                    # off-chain: zh = z*h, oz = 1-z   (both [128, 8] k-space)
                    zh_k = pr.tile([128, EC], BF16, tag="zh_k")
                    nc.vector.tensor_tensor(out=zh_k[:], in0=h_stat[:],
                                            in1=tpzv[:, :, 0], op=ALU.mult)
                    oz_k = pr.tile([128, EC], BF16, tag="oz_k")
                    nc.scalar.activation(oz_k[:], tpzv[:, :, 0], AF.Identity,
                                         bias=1.0, scale=-1.0)
                    # chain: tanh -> m1 = (1-z)*n -> h' = m1 + zh
                    n_k = pr.tile([128, EC], BF16, tag="n_k")
                    nc.scalar.activation(n_k[:], tpnv[:, :, 0], AF.Tanh)
                    m1 = pr.tile([128, EC], BF16, tag="m1")
                    nc.vector.tensor_tensor(out=m1[:], in0=oz_k[:], in1=n_k[:],
                                            op=ALU.mult)
                    nc.vector.tensor_tensor(out=h_stat[:], in0=m1[:], in1=zh_k[:],
                                            op=ALU.add)
                    nc.scalar.activation(htv[:, :, t:t + 1],
                                         h_stat[:].unsqueeze(2), AF.Relu)

                # ------ phase 5: logits = relu(H) @ out_w^T + out_b ------
                owtv = owt_sb[:].rearrange("p (k v) -> p k v", k=EC)
                with tc.tile_pool(name="fin", bufs=2) as pf, \
                     tc.tile_pool(name="finps", bufs=2, space="PSUM") as pfps:
                    for vb in range(VP // 512):
                        ops = pfps.tile([T, 512], F32, space="PSUM", tag="ops")
                        for k in range(EC):
                            nc.tensor.matmul(ops[:T, :], lhsT=htv[:, k, :],
                                             rhs=owtv[:, k, 512 * vb:512 * (vb + 1)],
                                             start=(k == 0), stop=False)
                        nc.tensor.matmul(ops[:T, :], lhsT=ones_bf[:1, :T],
                                         rhs=outb_sb[:1, 512 * vb:512 * (vb + 1)],
                                         start=False, stop=True)
                        osb = pf.tile([T, 512], F32, tag="osb")
                        nc.vector.tensor_copy(osb[:], ops[:T, :])
                        nc.sync.dma_start(out_d[:, 512 * vb:512 * (vb + 1)], osb[:])

    nc.compile()
    return nc


def _prep_inputs(inp):
    idx_enc = np.concatenate([inp["input_diagnosis"], inp["input_procedure"],
                              inp["input_medicine"]]).astype(np.int64)
    tokens = np.concatenate([np.array([V0], np.int64),
                             inp["dec_tokens"].astype(np.int64)])
    enc_emb = np.asarray(inp["enc_emb"], np.float32)
    dec_emb = np.asarray(inp["dec_emb"], np.float32)

    ctx = np.ascontiguousarray(enc_emb[idx_enc])                       # [320, 1024]
    decx = np.ascontiguousarray(dec_emb[tokens])                       # [65, 1024]
    we = np.ascontiguousarray(np.asarray(inp["attn_w"], np.float32)[0, E:]).reshape(1, E)

    w_ih = np.asarray(inp["gru_w_ih"], np.float32)                     # [3072, 2048]
    w_hh = np.asarray(inp["gru_w_hh"], np.float32)                     # [3072, 1024]
    b_ih = np.asarray(inp["gru_b_ih"], np.float32)
    b_hh = np.asarray(inp["gru_b_hh"], np.float32)
    assert not np.any(b_hh[2 * E:]), "nonzero b_hh n-gate not supported on device"

    whh_arr = _arrange_whh(w_hh).astype(NP_BF16)                       # [128, 24576]
    wc_arr = _arrange_w(np.ascontiguousarray(w_ih[:, :E])).astype(NP_BF16)
    wx_arr = _arrange_w(np.ascontiguousarray(w_ih[:, E:])).astype(NP_BF16)
    bias = b_ih.copy()
    bias[:2 * E] += b_hh[:2 * E]
    bias_arr = _bias_tall(bias)                                        # [128, 768] f32

    out_w = np.asarray(inp["out_w"], np.float32)
    out_b = np.asarray(inp["out_b"], np.float32)
    owp = np.zeros((NCORES * VP, E), np.float32)
    owp[:V] = out_w
    obp = np.zeros(NCORES * VP, np.float32)
    obp[:V] = out_b

    # h0 in k-order columns: h0k[p, 4*half + jh] = decx[0, 256*jh + 128*half + p]
    h0k = np.ascontiguousarray(
        np.transpose(decx[0].reshape(4, 2, 128), (2, 1, 0))).reshape(128, EC).astype(NP_BF16)

    base = {"ctx": ctx, "decx": decx, "we": we, "whh": whh_arr,
            "wc": wc_arr, "wx": wx_arr, "bias": bias_arr, "h0k": h0k}
    in_maps = []
    for i in range(NCORES):
        s = owp[i * VP:(i + 1) * VP]                                   # [4096, 1024]
        # owt[p, k*VP + v] = s[v, 256*(k%4) + 128*(k//4) + p]
        x = s.reshape(VP, 4, 2, 128)                                   # v, jh, half, p
        owt = np.ascontiguousarray(
            np.transpose(x, (3, 2, 1, 0))).astype(NP_BF16).reshape(128, EC * VP)
        m = dict(base)
        m["owt"] = owt
        m["outb"] = np.ascontiguousarray(
            obp[i * VP:(i + 1) * VP]).reshape(1, VP).astype(NP_BF16)
        in_maps.append(m)
    return in_maps


def kernel(**inputs):
    if "nc" not in _CACHE:
        _CACHE["nc"] = build_program()
    nc = _CACHE["nc"]
    in_maps = _prep_inputs({k: np.asarray(v) for k, v in inputs.items()})
    res = run_bass_kernel_spmd(nc, in_maps, core_ids=list(range(NCORES)))
    slices = [res.results[i]["out"] for i in range(NCORES)]            # each [65, 4096]
    logits = np.concatenate(slices, axis=1)[:, :V]
    return np.ascontiguousarray(logits.astype(np.float32))
